# revision 32
# baseline (speedup 1.0000x reference)
"""RWKV5 block, sequence-parallel across 8 trn2 cores.

Core c -> batch c//2, sequence half c%2 (tokens t0 = half*1024, TL=1024
= 2 recurrence chunks of TC=512). Each core runs FULL-width GEMMs
(C=2048, DF=7168) on its token half; every weight is streamed from HBM
once (Wv twice). Cross-core traffic per pair: one 512KB state AllGather
(recurrent state after chunk 1 -> second half) plus an 8KB x' halo
column AllGather for the ChannelMix time-shift.

Layout: activations channel-major [C, T]. v kept time-major [T, C]
(VT) for the attention a@v and k^T@v contractions.

v2: scheduling-focused rewrite. Attention decay masks are built as
column-shifted views of one per-head exp table (M) plus a diagonal
block tile (D), pa2/pout matmuls are column-restricted to the nonzero
mask region, groupnorm is deferred into a batched per-chunk pass with
gpsimd partition-broadcasts (no fp32 matmuls, no per-pair table
swaps), LN stats run on bf16 operands, and the final FFN quarter
writes y directly.
"""
import numpy as np
import concourse.bass as bass
import concourse.mybir as mybir
import concourse.tile as tile
from concourse import bacc
from concourse.masks import make_identity

f32 = mybir.dt.float32
bf16 = mybir.dt.bfloat16
AOT = mybir.AluOpType
AFT = mybir.ActivationFunctionType

C = 2048
H = 32         # heads
S = 64         # head dim
TC = 512       # recurrence chunk
TL = 1024      # local tokens per core
NCH = TL // TC # 2 local chunks
DF = 7168
P = 128
NK = C // P    # 16 channel chunks
NP = H // 2    # 16 head pairs
NJ = DF // P   # 56
NQ = 4         # DF quarters
JQ = NJ // NQ  # 14 j-chunks per quarter
EPS = 1e-5
HS_DIV = float(np.sqrt(S))
GROUPS = [[0, 1], [2, 3], [4, 5], [6, 7]]
TS = TL // TC  # 2 column sub-ranges of 512


def build_nc():
    nc = bacc.Bacc("TRN2", target_bir_lowering=False, debug=False, num_devices=8)
    dp = nc.declare_dram_parameter
    params = {
        "xT": dp("xT", [C, 1 + TL], f32, isOutput=False),
        # bf16 pre-cast copy for LN1: col 0 unused, col 1 halo, cols 2:2+TL
        # data (so 512-col slices start 4B-aligned for DVE 2x mode)
        "xTb": dp("xTb", [C, 2 + TL], bf16, isOutput=False),
        # weights pre-tiled on host: cols ordered (m-group, k, col-in-tile)
        "wr_t": dp("wr_t", [P, C * C // P], bf16, isOutput=False),
        "wk_t": dp("wk_t", [P, C * C // P], bf16, isOutput=False),
        "wv_t": dp("wv_t", [P, C * C // P], bf16, isOutput=False),
        "wo_t": dp("wo_t", [P, C * C // P], bf16, isOutput=False),
        "wfk_t": dp("wfk_t", [P, C * DF // P], bf16, isOutput=False),
        "wfv_t": dp("wfv_t", [P, C * DF // P], bf16, isOutput=False),
        "wfr_t": dp("wfr_t", [P, C * C // P], bf16, isOutput=False),
        "wspp": dp("wspp", [P, NP], f32, isOutput=False),
        "lnwpp": dp("lnwpp", [P, NP], f32, isOutput=False),
        "smask": dp("smask", [1, 1], f32, isOutput=False),
        "tdv": dp("tdv", [1, H], f32, isOutput=False),
        "uv": dp("uv", [1, H], f32, isOutput=False),
        "yT": dp("yT", [C, TL], f32, isOutput=True),
    }
    for nm, cols in [("ln1g", NK), ("ln1b", NK), ("ln2g", NK), ("ln2b", NK),
                     ("mxk", NK), ("mxv", NK), ("mxr", NK), ("fmk", NK),
                     ("fmr", NK), ("lnxg", NP), ("lnxb", NP)]:
        params[nm] = dp(nm, [P, cols], f32, isOutput=False)
    with tile.TileContext(nc) as tc:
        _build(nc, tc, params)
    nc.compile()
    return nc


def _build(nc, tc, params):
    ctxs = []

    def pool(name, bufs, space="SBUF"):
        p = tc.tile_pool(name=name, bufs=bufs, space=space)
        ctxs.append(p)
        return p.__enter__()

    const = pool("const", 1)
    pers = pool("pers", 1)
    big = pool("big", 65)          # [P,1+TL]-bf16-slab activation tiles
    scr = pool("scr", 9)           # [P,2TC]-slab scratch
    xsrc = pool("xsrc", 3)         # [P,1+TL]-f32 streamed sources
    sscr = pool("sscr", 10)        # small [P,S] scratch
    mtb = pool("mtb", 4)           # [P,TC]-bf16 per-head decay masks WD
    wts = pool("wts", 3)           # [128,2048]bf16 weight-blob ring
    psa = pool("psa", 3, space="PSUM")   # [P,2TC] f32 (2 banks)
    psb = pool("psb", 2, space="PSUM")   # [P,TC] f32 (1 bank)
    drm = pool("drm", 1, space="DRAM")

    cnt = [0]

    def bigt(dtype=bf16, cols=1 + TL):
        cnt[0] += 1
        return big.tile([P, cols], dtype, tag="big", name=f"b_{cnt[0]}")

    def sc(shape=(P, TC), dtype=f32):
        cnt[0] += 1
        return scr.tile(list(shape), dtype, tag="scr", name=f"sc_{cnt[0]}")

    def xsc():
        cnt[0] += 1
        return xsrc.tile([P, 1 + TL], f32, tag="xsrc", name=f"xs_{cnt[0]}")

    def ssc(shape=(P, S), dtype=f32):
        cnt[0] += 1
        return sscr.tile(list(shape), dtype, tag="sscr", name=f"ss_{cnt[0]}")

    def mt_tile():
        cnt[0] += 1
        return mtb.tile([P, TC], bf16, tag="mtb", name=f"mt_{cnt[0]}")

    def wt_tile(cols=2048):
        cnt[0] += 1
        return wts.tile([P, cols], bf16, tag="wt", name=f"wt_{cnt[0]}")

    def psa_():
        cnt[0] += 1
        return psa.tile([P, 2 * TC], f32, tag="psa", name=f"pa_{cnt[0]}")

    def pst_(shape=(P, TC), dtype=f32):
        cnt[0] += 1
        return psb.tile(list(shape), dtype, tag="psb", name=f"pb_{cnt[0]}")

    # x (bf16 pre-cast) DMAs issued first so they overlap const building
    XB = [bigt(cols=2 + TL) for _ in range(NK)]
    for k in range(NK):
        nc.sync.dma_start(XB[k][:], params["xTb"][k * P:(k + 1) * P, :])

    # ---------------- constants ----------------
    IOTA_T = const.tile([P, TC], f32, tag="iota_t")
    nc.gpsimd.iota(IOTA_T[:], pattern=[[1, TC]], base=0, channel_multiplier=0,
                   allow_small_or_imprecise_dtypes=True)
    IDENT = const.tile([P, P], bf16, tag="ident")
    make_identity(nc, IDENT[:])
    ONES_KB = const.tile([P, 1], bf16, tag="ones_kb")
    nc.gpsimd.memset(ONES_KB[:], 1.0)
    # GSEL[ch, c]: rows 0:64 set at col 30, rows 64:128 at col 31. Slicing
    # GSEL[:, 30-2p : 62-2p] yields a [128, 32] selector whose matmul
    # accumulates pair p's per-head column sums into rows 2p:2p+2.
    GSEL = const.tile([P, S - 2], bf16, tag="gsel")
    nc.gpsimd.memset(GSEL[:], 0.0)
    nc.gpsimd.memset(GSEL[0:S, 30:31], 1.0)
    nc.gpsimd.memset(GSEL[S:P, 31:32], 1.0)
    # IOTAW[p, j*64+c] = 511 - 128*j - p  (contrib decay exponents)
    IOTAW = const.tile([P, 4 * S], f32, tag="iotaw")
    nc.gpsimd.iota(IOTAW[:], pattern=[[-P, 4], [0, S]], base=TC - 1,
                   channel_multiplier=-1, allow_small_or_imprecise_dtypes=True)
    EPSB = const.tile([P, 1], f32, tag="epsb")
    nc.gpsimd.memset(EPSB[:], EPS)
    # IOTA_WD[p, x] = x - p - 1 where x > p else +1e30. exp(lnw * .) gives
    # the decay mask w^(x-1-p) as a function of x = t - jP, valid for every
    # 128-token block j (diag u term added separately on cols 0:128).
    IOTA_WD = const.tile([P, TC], f32, tag="iota_wd")
    iwd_raw = sc()
    nc.gpsimd.iota(iwd_raw[:], pattern=[[1, TC]], base=-1, channel_multiplier=-1,
                   allow_small_or_imprecise_dtypes=True)
    nc.gpsimd.affine_select(IOTA_WD[:], iwd_raw[:], pattern=[[1, TC]], base=-1,
                            channel_multiplier=-1, compare_op=AOT.is_ge,
                            fill=1e30)

    def ld(name, cols):
        t = const.tile([P, cols], f32, tag=name, name=name)
        nc.sync.dma_start(t[:], params[name][:])
        return t

    LN1G = ld("ln1g", NK); LN1B = ld("ln1b", NK)
    LN2G = ld("ln2g", NK); LN2B = ld("ln2b", NK)
    MXK = ld("mxk", NK); MXV = ld("mxv", NK); MXR = ld("mxr", NK)
    FMK = ld("fmk", NK); FMR = ld("fmr", NK)
    LNXG = ld("lnxg", NP); LNXB = ld("lnxb", NP)
    WSPP = ld("wspp", NP)
    LNWPP = ld("lnwpp", NP)

    def onem(src, name):
        t = const.tile([P, NK], f32, tag=name, name=name)
        nc.vector.tensor_scalar(t[:], src[:], -1.0, 1.0, AOT.mult, AOT.add)
        return t
    MXK1 = onem(MXK, "mxk1"); MXV1 = onem(MXV, "mxv1"); MXR1 = onem(MXR, "mxr1")
    FMK1 = onem(FMK, "fmk1"); FMR1 = onem(FMR, "fmr1")

    TD = const.tile([P, H], f32, tag="td")
    nc.sync.dma_start(TD[:], params["tdv"][0:1, :].partition_broadcast(P))
    UU = const.tile([P, H], f32, tag="uu")
    nc.sync.dma_start(UU[:], params["uv"][0:1, :].partition_broadcast(P))
    SMB = const.tile([P, 1], f32, tag="smb")
    nc.sync.dma_start(SMB[:], params["smask"][0:1, :].partition_broadcast(P))
    NEGLNW = const.tile([P, H], f32, tag="neglnw")
    nc.scalar.activation(NEGLNW[:], TD[:], AFT.Exp)
    LNW = const.tile([P, H], f32, tag="lnw")
    nc.vector.tensor_scalar_mul(LNW[:], NEGLNW[:], -1.0)

    xT = params["xT"]; yT = params["yT"]

    # DRAM tiles: collectives + x' spill + groupnorm broadcast bounce
    sout_d = drm.tile([P, NP * S], f32, tag="soutd")
    sgat_d = drm.tile([2 * P, NP * S], f32, tag="sgatd")
    xcol_d = drm.tile([P, NK], f32, tag="xcold")
    xcgat_d = drm.tile([2 * P, NK], f32, tag="xcgatd")
    xprime_d = drm.tile([C, TL], bf16, tag="xprd")
    rs_d = [drm.tile([H, TC], bf16, tag=f"rsd_{i}", name=f"rsd_{i}")
            for i in range(NCH)]
    mr_d = [drm.tile([H, TC], bf16, tag=f"mrd_{i}", name=f"mrd_{i}")
            for i in range(NCH)]

    # ---------- layernorm over channel dim (bf16, ts-pipelined) ----------
    def ln_pass(src_main, src_halo, g, b, dst_tiles, halo_mask,
                post_norm=None):
        """src_main(k, ts) -> [P, TC] bf16 aligned AP; src_halo(k) -> [P, 1]
        bf16 AP. Writes normalized bf16 into dst_tiles[k] ([P, 2+TL]: halo
        at col 1, main at cols 2:2+TL). ts=0 stats/chain/normalize issue
        before ts=1 stats so a ts-major GEMM can start on ts=0 columns
        while ts=1 normalizes. post_norm(k, ts) issues per-k mixes after
        each normalize. Ln/Exp batched (2 table loads per batch); the halo
        is chained with ts=1 so its AllGather (LN2) is covered."""
        pssA = psa_(); psqA = psa_()

        def stats(ts):
            for k in range(NK):
                s = src_main(k, ts)
                sq = sc((P, TC), bf16)
                nc.scalar.square(sq[:], s)
                nc.tensor.matmul(pssA[0:1, ts * TC:(ts + 1) * TC], ONES_KB[:],
                                 s, start=(k == 0), stop=(k == NK - 1))
                nc.tensor.matmul(psqA[0:1, ts * TC:(ts + 1) * TC], ONES_KB[:],
                                 sq[:], start=(k == 0), stop=(k == NK - 1))

        def chain_mv(pss, psq, n):
            m_ = sc((1, n)); nc.scalar.mul(m_[:], pss, 1.0 / C)
            q_ = sc((1, n)); nc.scalar.mul(q_[:], psq, 1.0 / C)
            msq = sc((1, n)); nc.vector.tensor_mul(msq[:], m_[:], m_[:])
            var = sc((1, n)); nc.vector.tensor_sub(var[:], q_[:], msq[:])
            return m_, var

        def chain_fin(items):
            # batched: all Ln, then all Exp (one table load each)
            lnvs = []
            for m_, var, n in items:
                lnv = sc((1, n))
                nc.scalar.activation(lnv[:], var[:], AFT.Ln,
                                     bias=EPSB[0:1, 0:1])
                lnvs.append(lnv)
            outs = []
            for (m_, var, n), lnv in zip(items, lnvs):
                rs = sc((1, n))
                nc.scalar.activation(rs[:], lnv[:], AFT.Exp, scale=-0.5)
                mrs = sc((1, n))
                nc.vector.scalar_tensor_tensor(mrs[:], m_[:], -1.0, rs[:],
                                               AOT.mult, AOT.mult)
                rsb = sc((1, n), bf16); nc.vector.tensor_copy(rsb[:], rs[:])
                mrb = sc((1, n), bf16); nc.vector.tensor_copy(mrb[:], mrs[:])
                brs = sc((P, n), bf16)
                nc.gpsimd.partition_broadcast(brs[:], rsb[:])
                bmrs = sc((P, n), bf16)
                nc.gpsimd.partition_broadcast(bmrs[:], mrb[:])
                outs.append((brs, bmrs))
            return outs

        def norm(k, ts, brs, bmrs):
            dst = dst_tiles[k]
            tmp = sc((P, TC), bf16)
            nc.vector.tensor_mul(tmp[:], src_main(k, ts), brs[:])
            nc.vector.tensor_add(tmp[:], tmp[:], bmrs[:])
            nc.vector.tensor_scalar(dst[:, 2 + ts * TC:2 + (ts + 1) * TC],
                                    tmp[:], g[:, k:k + 1], b[:, k:k + 1],
                                    AOT.mult, AOT.add)

        stats(0)
        m0, v0 = chain_mv(pssA[0:1, 0:TC], psqA[0:1, 0:TC], TC)
        (bc0,) = chain_fin([(m0, v0, TC)])
        stats(1)
        pssh = pst_((1, 1)); psqh = pst_((1, 1))
        for k in range(NK):
            hs = src_halo(k)
            hsq = sc((P, 1), bf16)
            nc.scalar.square(hsq[:], hs)
            nc.tensor.matmul(pssh[:], ONES_KB[:], hs,
                             start=(k == 0), stop=(k == NK - 1))
            nc.tensor.matmul(psqh[:], ONES_KB[:], hsq[:],
                             start=(k == 0), stop=(k == NK - 1))
        m1, v1 = chain_mv(pssA[0:1, TC:2 * TC], psqA[0:1, TC:2 * TC], TC)
        mh, vh = chain_mv(pssh[:], psqh[:], 1)
        bc1, bch = chain_fin([(m1, v1, TC), (mh, vh, 1)])
        # ts=0 normalize (no halo dependence)
        for k in range(NK):
            norm(k, 0, *bc0)
        # halo normalize (before the ts=0 mixes, which read col 1)
        for k in range(NK):
            dst = dst_tiles[k]
            tmp = sc((P, 1), bf16)
            nc.vector.tensor_mul(tmp[:], src_halo(k), bch[0][:])
            nc.vector.tensor_add(tmp[:], tmp[:], bch[1][:])
            nc.vector.tensor_scalar(dst[:, 1:2], tmp[:], g[:, k:k + 1],
                                    b[:, k:k + 1], AOT.mult, AOT.add)
            if halo_mask:
                nc.vector.tensor_scalar(dst[:, 1:2], dst[:, 1:2],
                                        SMB[:, 0:1], None, AOT.mult)
        if post_norm is not None:
            for k in range(NK):
                post_norm(k, 0)
        for k in range(NK):
            norm(k, 1, *bc1)
            if post_norm is not None:
                post_norm(k, 1)

    def mixf(dst, h, cf, k):
        """dst[:, 0:TL] = h_shift + cf[k]*(h - h_shift), full width."""
        dh = sc((P, TL), bf16)
        nc.gpsimd.tensor_sub(dh[:], h[:, 2:2 + TL], h[:, 1:1 + TL])
        nc.vector.scalar_tensor_tensor(dst[:, 0:TL], dh[:], cf[:, k:k + 1],
                                       h[:, 1:1 + TL], AOT.mult, AOT.add)

    # ---------- GEMM helper: out[m] = sum_k w_tiled[.,m,k] ^T @ in[k] ----------
    def gemm_std(wt_dram, in_tiles, out_tiles, n_out, act=None, accum=False,
                 n_in=NK, G=8, col_base=0, post=None, pre=None,
                 ts_major=False):
        """Host-pretiled weights: blob cols ordered (mg, k, 256). m-groups
        of 2; per group ceil(n_in/G) blob DMAs of [128, G*256]; two 2-bank
        PSUM tiles (one per mi), double-buffered across groups.
        act: None -> copy; 'sigmoid'; 'sqrelu'. accum: add into out.
        post(m, pq): custom finalize; pre(mgl): prefetch hook."""
        nh = (n_in + G - 1) // G
        for mgl in range(n_out // 2):
            if pre is not None:
                pre(mgl)
            base = col_base + mgl * n_in * 2 * P
            wtl = []
            for hf in range(nh):
                kn = min(G, n_in - hf * G)
                w = wt_tile(kn * 2 * P)
                nc.sync.dma_start(
                    w[:], wt_dram[:, base + hf * G * 2 * P:
                                  base + (hf * G + kn) * 2 * P])
                wtl.append(w)
            pq = [psa_() for _ in range(2)]
            if ts_major:
                # all ts=0 matmuls before any ts=1: lets the GEMM start as
                # soon as ts=0 inputs exist (LN still normalizing ts=1)
                for ts in range(TS):
                    for k in range(n_in):
                        w = wtl[k // G]
                        co = (k % G) * 2 * P
                        for mi in range(2):
                            nc.tensor.matmul(
                                pq[mi][:, ts * TC:(ts + 1) * TC],
                                w[:, co + mi * P:co + (mi + 1) * P],
                                in_tiles[k][:, ts * TC:(ts + 1) * TC],
                                start=(k == 0), stop=(k == n_in - 1))
            else:
                for k in range(n_in):
                    w = wtl[k // G]
                    co = (k % G) * 2 * P
                    for mi in range(2):
                        for ts in range(TS):
                            nc.tensor.matmul(
                                pq[mi][:, ts * TC:(ts + 1) * TC],
                                w[:, co + mi * P:co + (mi + 1) * P],
                                in_tiles[k][:, ts * TC:(ts + 1) * TC],
                                start=(k == 0), stop=(k == n_in - 1))
            for mi in range(2):
                m = mgl * 2 + mi
                if post is not None:
                    post(m, pq[mi])
                    continue
                dst = out_tiles[m][:, 0:TL]
                src = pq[mi][0:P, 0:TL]
                if act == "sigmoid":
                    nc.scalar.activation(dst, src, AFT.Sigmoid)
                elif act == "sqrelu":
                    rl = xsc()
                    nc.scalar.activation(rl[:, 0:TL], src, AFT.Relu)
                    nc.vector.tensor_mul(dst, rl[:, 0:TL], rl[:, 0:TL])
                elif accum:
                    nc.vector.tensor_add(dst, dst, src)
                else:
                    nc.vector.tensor_copy(dst, src)

    # ================= phase 1: LN1 =================
    # x streamed once as a host-pre-cast bf16 copy (xTb); stats + normalize
    # both read it. K-mix issues inline per (k, ts) via post_norm.
    H1 = [bigt(cols=2 + TL) for _ in range(NK)]
    MQ = [bigt(cols=TL) for _ in range(NK)]

    def post1(k, ts):
        dh = sc((P, TC), bf16)
        nc.gpsimd.tensor_sub(dh[:], H1[k][:, 2 + ts * TC:2 + (ts + 1) * TC],
                             H1[k][:, 1 + ts * TC:1 + (ts + 1) * TC])
        nc.vector.scalar_tensor_tensor(MQ[k][:, ts * TC:(ts + 1) * TC],
                                       dh[:], MXK[:, k:k + 1],
                                       H1[k][:, 1 + ts * TC:1 + (ts + 1) * TC],
                                       AOT.mult, AOT.add)

    ln_pass(lambda k, ts: XB[k][:, 2 + ts * TC:2 + (ts + 1) * TC],
            lambda k: XB[k][:, 1:2], LN1G, LN1B, H1, halo_mask=True,
            post_norm=post1)
    XB = None

    # ================= phase 2: K / V projections =================
    KT = [bigt(cols=TL) for _ in range(NK)]
    gemm_std(params["wk_t"], MQ, KT, NK, ts_major=True)
    for k in range(NK):
        mixf(MQ[k], H1[k], MXV, k)
    # V transposed: VT[tslab (8 x 128 tokens)] as 2 tiles of [P, 1024] each.
    # wv_t blob cols ordered (cb, k, 512); stationary = MQ slab, moving = w.
    VT = [[bigt(cols=TL) for _ in range(2)] for _ in range(2 * 4)]
    for tg in range(2):       # t-slab groups of 4
        for cb in range(4):   # c_out banks of 512
            pv = [psa_() for _ in range(2)]
            for kq in range(4):
                w = wt_tile()
                nc.sync.dma_start(
                    w[:], params["wv_t"][:, (cb * NK + kq * 4) * TC:
                                         (cb * NK + kq * 4 + 4) * TC])
                for kk in range(4):
                    k = kq * 4 + kk
                    for ti in range(4):
                        tslab = tg * 4 + ti
                        nc.tensor.matmul(
                            pv[ti // 2][:, (ti % 2) * TC:(ti % 2 + 1) * TC],
                            MQ[k][:, tslab * P:(tslab + 1) * P],
                            w[:, kk * TC:(kk + 1) * TC],
                            start=(k == 0), stop=(k == NK - 1))
            for ti in range(4):
                nc.vector.tensor_copy(
                    VT[tg * 4 + ti][cb // 2][:, (cb % 2) * TC:(cb % 2 + 1) * TC],
                    pv[ti // 2][:, (ti % 2) * TC:(ti % 2 + 1) * TC])

    def vsl(i, j, h):
        """[P,S] value slice: chunk i, 128-token slab j, head h."""
        voff = h * S
        return VT[i * 4 + j][voff // TL][:, voff % TL:voff % TL + S]

    # ============ phase 3: state contributions + AllGather ============
    # K transposed per 128-token block with ONE full 128x128 transpose
    # covering both heads of the pair.
    CONTRIB0 = [pers.tile([P, S], f32, tag=f"c0_{p}", name=f"c0_{p}")
                for p in range(NP)]
    for p in range(NP):
        wkct = []
        for hh in range(2):
            h = 2 * p + hh
            t = sc((P, 4 * S), bf16)
            nc.scalar.activation(t[:], IOTAW[:], AFT.Exp,
                                 scale=LNW[:, h:h + 1])
            wkct.append(t)
        cts = []
        for i in range(NCH):
            ptrf = pst_((P, 4 * P), bf16)
            for j in range(4):
                nc.tensor.transpose(
                    ptrf[:, j * P:(j + 1) * P],
                    KT[p][:, i * TC + j * P:i * TC + (j + 1) * P],
                    IDENT[:])
            kw = sc((P, 4 * P), bf16)
            for j in range(4):
                for hh in range(2):
                    o = j * P + hh * S
                    nc.vector.tensor_mul(kw[:, o:o + S], ptrf[:, o:o + S],
                                         wkct[hh][:, j * S:(j + 1) * S])
            pst = pst_((P, S))
            for hh in range(2):
                h = 2 * p + hh
                pr = slice(hh * S, hh * S + S)
                for j in range(4):
                    nc.tensor.matmul(pst[pr, :], kw[:, j * P + hh * S:
                                                    j * P + hh * S + S],
                                     vsl(i, j, h),
                                     start=(j == 0), stop=(j == 3))
            if i == 0:
                nc.vector.tensor_copy(CONTRIB0[p][:], pst[:])
                cts.append(CONTRIB0[p])
            else:
                c1 = ssc()
                nc.vector.tensor_copy(c1[:], pst[:])
                cts.append(c1)
        so = ssc()
        nc.vector.scalar_tensor_tensor(so[:], cts[0][:], WSPP[:, p:p + 1],
                                       cts[1][:], AOT.mult, AOT.add)
        nc.sync.dma_start(sout_d[:, p * S:(p + 1) * S], so[:])
    nc.gpsimd.collective_compute("AllGather", AOT.bypass, replica_groups=GROUPS,
                                 ins=[sout_d.opt()], outs=[sgat_d.opt()])

    # ================= phase 4: R projection (overlaps AG) =================
    for k in range(NK):
        mixf(MQ[k], H1[k], MXR, k)
    H1 = None
    RT = [bigt(cols=TL) for _ in range(NK)]
    gemm_std(params["wr_t"], MQ, RT, NK)
    MQ = None

    # incoming state = smask * (rank0 shard of gather)
    SIN = pers.tile([P, NP * S], f32, tag="sin", name="sin")
    nc.sync.dma_start(SIN[:], sgat_d[0:P, :])
    nc.vector.tensor_scalar(SIN[:], SIN[:], SMB[:, 0:1], None, AOT.mult)

    # ================= phase 5: attention =================
    # Decay mask per head = column-shifted view of M_h[p,c]=w^(c-p) (c>=p)
    # plus diagonal-block D_h[p,q]=w^(q-p-1) (q>p) + u_h*I. Mask is zero
    # for t <= jP (except diag), so pa2/pout matmuls are restricted to
    # cols >= jP. Groupnorm stats are collected per (pair, chunk) into a
    # batched [64, TC] tile; one scalar chain per chunk; normalization
    # applied with gpsimd partition-broadcasts (no tensor-engine work).
    XA = [bigt(cols=TL) for _ in range(NK)]
    PSALL = {}

    def attn_pair(i, p, masks=None):
        h0, h1 = 2 * p, 2 * p + 1
        if masks is None:
            masks = build_masks(p)
        WD_ = masks
        wb = sc(dtype=bf16)
        nc.scalar.activation(wb[:], IOTA_T[:], AFT.Exp,
                             scale=LNWPP[:, p:p + 1])
        st_mm = ssc((P, S), bf16)
        if i == 0:
            nc.gpsimd.tensor_copy(st_mm[:], SIN[:, p * S:(p + 1) * S])
        else:
            stt = ssc()
            nc.vector.scalar_tensor_tensor(stt[:], SIN[:, p * S:(p + 1) * S],
                                           WSPP[:, p:p + 1], CONTRIB0[p][:],
                                           AOT.mult, AOT.add)
            nc.gpsimd.tensor_copy(st_mm[:], stt[:])
        rtw = sc(dtype=bf16)
        nc.vector.tensor_mul(rtw[:], RT[p][:, i * TC:(i + 1) * TC], wb[:])
        pout = pst_((P, TC))

        def pout_mms(j, ast):
            for hh in range(2):
                h = 2 * p + hh
                pr = slice(hh * S, hh * S + S)
                nc.tensor.matmul(pout[pr, j * P:TC], vsl(i, j, h),
                                 ast[:, hh * TC + j * P:(hh + 1) * TC],
                                 start=False, stop=(j == 3))

        prev = None
        for j in range(4):
            pa2 = psa_()
            for hh in range(2):
                pr = slice(hh * S, hh * S + S)
                nc.tensor.matmul(
                    pa2[:, hh * TC + j * P:(hh + 1) * TC],
                    KT[p][pr, i * TC + j * P:i * TC + (j + 1) * P],
                    RT[p][pr, i * TC + j * P:(i + 1) * TC],
                    start=True, stop=True)
            if j == 0:
                # state term (start=True clears the pout bank); issued
                # after the first pa2 so the tensor queue isn't blocked
                # waiting on wb/rtw from the scalar/vector engines.
                for hh in range(2):
                    pr = slice(hh * S, hh * S + S)
                    nc.tensor.matmul(pout[pr, :], st_mm[pr, :], rtw[pr, :],
                                     start=True, stop=False)
            ast = sc((P, 2 * TC), bf16)
            for hh in range(2):
                o = hh * TC + j * P
                nc.vector.tensor_mul(ast[:, o:(hh + 1) * TC],
                                     pa2[:, o:(hh + 1) * TC],
                                     WD_[hh][:, 0:TC - j * P])
            if prev is not None:
                pout_mms(*prev)
            prev = (j, ast)
        pout_mms(*prev)
        # raw attention out + gathered groupnorm stats (GSEL accumulate);
        # psum copy + square on the scalar engine (vector is the pacer here)
        nc.scalar.copy(XA[p][:, i * TC:(i + 1) * TC], pout[:])
        sq = sc((P, TC), bf16)
        nc.scalar.square(sq[:], XA[p][:, i * TC:(i + 1) * TC])
        if p == 0:
            PSALL[i] = psa_()
        sel = GSEL[:, 30 - 2 * p:62 - 2 * p]
        nc.tensor.matmul(PSALL[i][0:H, 0:TC], sel,
                         XA[p][:, i * TC:(i + 1) * TC],
                         start=(p == 0), stop=(p == NP - 1))
        nc.tensor.matmul(PSALL[i][0:H, TC:2 * TC], sel, sq[:],
                         start=(p == 0), stop=(p == NP - 1))

    def build_masks(p):
        WD_ = []
        for hh in range(2):
            h = 2 * p + hh
            wd = mt_tile()
            nc.scalar.activation(wd[:], IOTA_WD[:], AFT.Exp,
                                 scale=LNW[:, h:h + 1])
            # diagonal 'u' bonus lands in the first 128-col block
            nc.vector.scalar_tensor_tensor(wd[:, 0:P], IDENT[:],
                                           UU[:, h:h + 1], wd[:, 0:P],
                                           AOT.mult, AOT.add)
            WD_.append(wd)
        return WD_

    def gn_chain(i):
        ps = PSALL.pop(i)
        m_ = sc((H, TC))
        nc.scalar.mul(m_[:], ps[0:H, 0:TC], 1.0 / (S * HS_DIV))
        q_ = sc((H, TC))
        nc.scalar.mul(q_[:], ps[0:H, TC:2 * TC],
                      1.0 / (S * HS_DIV * HS_DIV))
        msq = sc((H, TC)); nc.vector.tensor_mul(msq[:], m_[:], m_[:])
        var = sc((H, TC)); nc.vector.tensor_sub(var[:], q_[:], msq[:])
        lnv = sc((H, TC))
        nc.scalar.activation(lnv[:], var[:], AFT.Ln, bias=EPSB[0:H, 0:1])
        rs = sc((H, TC))
        nc.scalar.activation(rs[:], lnv[:], AFT.Exp, scale=-0.5)
        mrs = sc((H, TC))
        nc.vector.scalar_tensor_tensor(mrs[:], m_[:], -1.0, rs[:],
                                       AOT.mult, AOT.mult)
        rsh = sc((H, TC))
        nc.vector.tensor_scalar_mul(rsh[:], rs[:], 1.0 / HS_DIV)
        rsb = sc((H, TC), bf16)
        nc.vector.tensor_copy(rsb[:], rsh[:])
        mrb = sc((H, TC), bf16)
        nc.vector.tensor_copy(mrb[:], mrs[:])
        nc.sync.dma_start(rs_d[i][:], rsb[:])
        nc.sync.dma_start(mr_d[i][:], mrb[:])

    def gn_finish(i, p):
        # per-head broadcast of the groupnorm scale/shift rows via a DMA
        # bounce through DRAM (engines can't read non-32-aligned rows).
        brs = sc(dtype=bf16)
        bmrs = sc(dtype=bf16)
        for hh in range(2):
            r = 2 * p + hh
            pr = slice(hh * S, hh * S + S)
            nc.sync.dma_start(brs[pr, :],
                              rs_d[i][r:r + 1, :].partition_broadcast(S))
            nc.sync.dma_start(bmrs[pr, :],
                              mr_d[i][r:r + 1, :].partition_broadcast(S))
        xa = sc(dtype=bf16)
        nc.gpsimd.tensor_mul(xa[:], XA[p][:, i * TC:(i + 1) * TC], brs[:])
        nc.gpsimd.tensor_add(xa[:], xa[:], bmrs[:])
        nc.vector.tensor_scalar(XA[p][:, i * TC:(i + 1) * TC], xa[:],
                                LNXG[:, p:p + 1], LNXB[:, p:p + 1],
                                AOT.mult, AOT.add)

    for p in range(NP):
        attn_pair(0, p)
    mk0 = build_masks(0)
    mk1 = build_masks(1)
    attn_pair(1, 0, mk0)
    attn_pair(1, 1, mk1)
    gn_chain(0)
    for p in range(2, NP):
        attn_pair(1, p)
        gn_finish(0, p - 2)
    gn_finish(0, NP - 2)
    gn_finish(0, NP - 1)
    gn_chain(1)
    for p in range(NP):
        gn_finish(1, p)
    RT = KT = VT = None

    # ================= phase 6: Wo + residual, spill x' =================
    XP = [bigt(cols=TL) for _ in range(NK)]
    xr_tiles = {}

    def wo_pre(mgl):
        for m in (2 * mgl, 2 * mgl + 1):
            t = xsc()
            nc.sync.dma_start(t[:], xT[m * P:(m + 1) * P, :])
            xr_tiles[m] = t

    def wo_post(m, pq):
        xr = xr_tiles.pop(m)
        for ts in range(TS):
            c0, c1 = ts * TC, (ts + 1) * TC
            nc.vector.tensor_add(XP[m][:, c0:c1], pq[0:P, c0:c1],
                                 xr[:, 1 + c0:1 + c1])
            nc.sync.dma_start(xprime_d[m * P:(m + 1) * P, c0:c1],
                              XP[m][:, c0:c1])
        lc = sc((P, 1))
        nc.vector.tensor_copy(lc[:], XP[m][:, TL - 1:TL])
        nc.sync.dma_start(xcol_d[:, m:m + 1], lc[:])

    gemm_std(params["wo_t"], XA, XP, NK, post=wo_post, pre=wo_pre)
    XA = None
    nc.gpsimd.collective_compute("AllGather", AOT.bypass, replica_groups=GROUPS,
                                 ins=[xcol_d.opt()], outs=[xcgat_d.opt()])

    # ================= phase 7: LN2 + mixes =================
    XCH = const.tile([P, NK], f32, tag="xch")   # per-chunk halo cols
    nc.sync.dma_start(XCH[:], xcgat_d[0:P, :])
    XCHB = const.tile([P, NK], bf16, tag="xchb")
    nc.vector.tensor_copy(XCHB[:], XCH[:])

    H2 = [bigt(cols=2 + TL) for _ in range(NK)]
    MFK = [bigt(cols=TL) for _ in range(NK)]
    MFR = [bigt(cols=TL) for _ in range(NK)]

    def post2(k, ts):
        dh = sc((P, TC), bf16)
        nc.gpsimd.tensor_sub(dh[:], H2[k][:, 2 + ts * TC:2 + (ts + 1) * TC],
                             H2[k][:, 1 + ts * TC:1 + (ts + 1) * TC])
        hsv = H2[k][:, 1 + ts * TC:1 + (ts + 1) * TC]
        nc.vector.scalar_tensor_tensor(MFK[k][:, ts * TC:(ts + 1) * TC],
                                       dh[:], FMK[:, k:k + 1], hsv,
                                       AOT.mult, AOT.add)
        nc.vector.scalar_tensor_tensor(MFR[k][:, ts * TC:(ts + 1) * TC],
                                       dh[:], FMR[:, k:k + 1], hsv,
                                       AOT.mult, AOT.add)

    ln_pass(lambda k, ts: XP[k][:, ts * TC:(ts + 1) * TC],
            lambda k: XCHB[:, k:k + 1], LN2G, LN2B, H2, halo_mask=True,
            post_norm=post2)
    XP = None
    H2 = None

    # ================= phase 8: gate = sigmoid(mfr @ wfr) =================
    GT = [bigt(cols=TL) for _ in range(NK)]
    gemm_std(params["wfr_t"], MFR, GT, NK, act="sigmoid", ts_major=True)
    MFR = None

    # ========== phase 9: FFN quarters: kf=relu^2(mfk@wfk); kv+=wfv^T@kf ==========
    # last quarter's Wfv finalize writes y = x' + gate*kv directly.
    KV = [bigt(cols=TL) for _ in range(NK)]
    xp_tiles = {}

    def y_pre(mgl):
        for m in (2 * mgl, 2 * mgl + 1):
            t = sc((P, TL), bf16)
            nc.sync.dma_start(t[:], xprime_d[m * P:(m + 1) * P, :])
            xp_tiles[m] = t

    def y_post(m, pq):
        xp = xp_tiles.pop(m)
        for ts in range(TS):
            c0, c1 = ts * TC, (ts + 1) * TC
            kvf = sc()
            nc.vector.tensor_add(kvf[:], KV[m][:, c0:c1], pq[0:P, c0:c1])
            gk = sc()
            nc.vector.tensor_mul(gk[:], GT[m][:, c0:c1], kvf[:])
            yo = sc()
            nc.vector.tensor_add(yo[:], xp[:, c0:c1], gk[:])
            nc.sync.dma_start(yT[m * P:(m + 1) * P, c0:c1], yo[:])

    for q in range(NQ):
        KF = [bigt(cols=TL) for _ in range(JQ)]
        gemm_std(params["wfk_t"], MFK, KF, JQ, act="sqrelu",
                 col_base=q * (JQ // 2) * NK * 2 * P)
        # kv partial: contract the quarter's 14 j-chunks
        if q < NQ - 1:
            gemm_std(params["wfv_t"], KF, KV, NK, accum=(q > 0), n_in=JQ, G=7,
                     col_base=q * (NK // 2) * JQ * 2 * P)
        else:
            gemm_std(params["wfv_t"], KF, KV, NK, n_in=JQ, G=7,
                     col_base=q * (NK // 2) * JQ * 2 * P,
                     post=y_post, pre=y_pre)
        KF = None
    MFK = None

    for c in reversed(ctxs):
        c.__exit__(None, None, None)


# ----------------------------------------------------------------------
# Host-side sharding / gather
# ----------------------------------------------------------------------
import ml_dtypes

_NC_CACHE = {}


def _vec_pk(v, nk=NK):
    return np.ascontiguousarray(np.asarray(v).reshape(nk, P).T.astype(np.float32))


def _make_in_maps(inputs):
    x = np.asarray(inputs["x"], np.float32)
    bf = ml_dtypes.bfloat16
    td = np.asarray(inputs["time_decay"], np.float64)
    w = np.exp(-np.exp(td))                      # [H]
    ws = w ** TC
    wspp = np.zeros((P, NP), np.float32)
    lnwpp = np.zeros((P, NP), np.float32)
    lnw = -np.exp(td)
    for p in range(NP):
        wspp[0:S, p] = ws[2 * p]
        wspp[S:P, p] = ws[2 * p + 1]
        lnwpp[0:S, p] = lnw[2 * p]
        lnwpp[S:P, p] = lnw[2 * p + 1]
    def _tile_mk(W):
        """[n_in*128, n_mg*256] -> [128, n_mg*n_in*256], cols (mg, k, c)."""
        n_in = W.shape[0] // P
        n_mg = W.shape[1] // (2 * P)
        return np.ascontiguousarray(
            W.reshape(n_in, P, n_mg, 2 * P).transpose(1, 2, 0, 3)
             .reshape(P, -1).astype(bf))

    wcache = {}
    for nm, key in [("wr_t", "Wr"), ("wk_t", "Wk"), ("wo_t", "Wo"),
                    ("wfr_t", "Wfr"), ("wfk_t", "Wfk")]:
        wcache[nm] = _tile_mk(np.asarray(inputs[key], np.float32))
    Wfv = np.asarray(inputs["Wfv"], np.float32)
    wcache["wfv_t"] = np.ascontiguousarray(np.concatenate(
        [_tile_mk(Wfv[q * JQ * P:(q + 1) * JQ * P, :]) for q in range(NQ)],
        axis=1))
    Wv = np.asarray(inputs["Wv"], np.float32)
    wcache["wv_t"] = np.ascontiguousarray(
        Wv.reshape(NK, P, 4, TC).transpose(1, 2, 0, 3).reshape(P, -1).astype(bf))
    maps = []
    for c in range(8):
        b, half = c // 2, c % 2
        t0 = half * TL
        xh = np.zeros((C, 1 + TL), np.float32)
        xh[:, 1:] = x[b, t0:t0 + TL, :].T
        if half == 1:
            xh[:, 0] = x[b, t0 - 1, :]
        xhb = np.zeros((C, 2 + TL), np.float32)
        xhb[:, 1:] = xh
        maps.append({
            "xT": np.ascontiguousarray(xh),
            "xTb": np.ascontiguousarray(xhb.astype(bf)),
            **wcache,
            "wspp": wspp, "lnwpp": lnwpp,
            "smask": np.full((1, 1), float(half), np.float32),
            "ln1g": _vec_pk(inputs["ln1_g"]), "ln1b": _vec_pk(inputs["ln1_b"]),
            "ln2g": _vec_pk(inputs["ln2_g"]), "ln2b": _vec_pk(inputs["ln2_b"]),
            "mxk": _vec_pk(inputs["att_mix_k"]), "mxv": _vec_pk(inputs["att_mix_v"]),
            "mxr": _vec_pk(inputs["att_mix_r"]),
            "fmk": _vec_pk(inputs["ffn_mix_k"]), "fmr": _vec_pk(inputs["ffn_mix_r"]),
            "lnxg": _vec_pk(inputs["lnx_g"], NP),
            "lnxb": _vec_pk(inputs["lnx_b"], NP),
            "tdv": np.ascontiguousarray(np.asarray(inputs["time_decay"],
                                                   np.float32)[None, :]),
            "uv": np.ascontiguousarray(np.asarray(inputs["time_faaaa"],
                                                  np.float32)[None, :]),
        })
    return maps


def run_on_hw(inputs, trace=False):
    from concourse.bass_utils import run_bass_kernel_spmd
    if "nc" not in _NC_CACHE:
        _NC_CACHE["nc"] = build_nc()
    nc = _NC_CACHE["nc"]
    maps = _make_in_maps(inputs)
    res = run_bass_kernel_spmd(nc, maps, core_ids=list(range(8)), trace=trace)
    B = 4
    out = np.zeros((B, 2 * TL, C), np.float32)
    for c in range(8):
        b, half = c // 2, c % 2
        out[b, half * TL:(half + 1) * TL, :] = res.results[c]["yT"].T
    return out, res


def kernel(**inputs) -> np.ndarray:
    out, _ = run_on_hw(inputs, trace=False)
    return out


# revision 33
# speedup vs baseline: 1.0289x; 1.0289x over previous
"""RWKV5 block, sequence-parallel across 8 trn2 cores.

Core c -> batch c//2, sequence half c%2 (tokens t0 = half*1024, TL=1024
= 2 recurrence chunks of TC=512). Each core runs FULL-width GEMMs
(C=2048, DF=7168) on its token half; every weight is streamed from HBM
once (Wv twice). Cross-core traffic per pair: one 512KB state AllGather
(recurrent state after chunk 1 -> second half) plus an 8KB x' halo
column AllGather for the ChannelMix time-shift.

Layout: activations channel-major [C, T]. v kept time-major [T, C]
(VT) for the attention a@v and k^T@v contractions.

v2: scheduling-focused rewrite. Attention decay masks are built as
column-shifted views of one per-head exp table (M) plus a diagonal
block tile (D), pa2/pout matmuls are column-restricted to the nonzero
mask region, groupnorm is deferred into a batched per-chunk pass with
gpsimd partition-broadcasts (no fp32 matmuls, no per-pair table
swaps), LN stats run on bf16 operands, and the final FFN quarter
writes y directly.
"""
import numpy as np
import concourse.bass as bass
import concourse.mybir as mybir
import concourse.tile as tile
from concourse import bacc
from concourse.masks import make_identity

f32 = mybir.dt.float32
bf16 = mybir.dt.bfloat16
AOT = mybir.AluOpType
AFT = mybir.ActivationFunctionType

C = 2048
H = 32         # heads
S = 64         # head dim
TC = 512       # recurrence chunk
TL = 1024      # local tokens per core
NCH = TL // TC # 2 local chunks
DF = 7168
P = 128
NK = C // P    # 16 channel chunks
NP = H // 2    # 16 head pairs
NJ = DF // P   # 56
NQ = 4         # DF quarters
JQ = NJ // NQ  # 14 j-chunks per quarter
EPS = 1e-5
HS_DIV = float(np.sqrt(S))
GROUPS = [[0, 1], [2, 3], [4, 5], [6, 7]]
TS = TL // TC  # 2 column sub-ranges of 512


def build_nc():
    nc = bacc.Bacc("TRN2", target_bir_lowering=False, debug=False, num_devices=8)
    dp = nc.declare_dram_parameter
    params = {
        "xT": dp("xT", [C, 1 + TL], f32, isOutput=False),
        # bf16 pre-cast copy for LN1: col 0 unused, col 1 halo, cols 2:2+TL
        # data (so 512-col slices start 4B-aligned for DVE 2x mode)
        "xTb": dp("xTb", [C, 2 + TL], bf16, isOutput=False),
        # weights pre-tiled on host: cols ordered (m-group, k, col-in-tile)
        "wr_t": dp("wr_t", [P, C * C // P], bf16, isOutput=False),
        "wk_t": dp("wk_t", [P, C * C // P], bf16, isOutput=False),
        "wv_t": dp("wv_t", [P, C * C // P], bf16, isOutput=False),
        "wo_t": dp("wo_t", [P, C * C // P], bf16, isOutput=False),
        "wfk_t": dp("wfk_t", [P, C * DF // P], bf16, isOutput=False),
        "wfv_t": dp("wfv_t", [P, C * DF // P], bf16, isOutput=False),
        "wfr_t": dp("wfr_t", [P, C * C // P], bf16, isOutput=False),
        "wspp": dp("wspp", [P, NP], f32, isOutput=False),
        "lnwpp": dp("lnwpp", [P, NP], f32, isOutput=False),
        "smask": dp("smask", [1, 1], f32, isOutput=False),
        "tdv": dp("tdv", [1, H], f32, isOutput=False),
        "uv": dp("uv", [1, H], f32, isOutput=False),
        "yT": dp("yT", [C, TL], f32, isOutput=True),
    }
    for nm, cols in [("ln1g", NK), ("ln1b", NK), ("ln2g", NK), ("ln2b", NK),
                     ("mxk", NK), ("mxv", NK), ("mxr", NK), ("fmk", NK),
                     ("fmr", NK), ("lnxg", NP), ("lnxb", NP)]:
        params[nm] = dp(nm, [P, cols], f32, isOutput=False)
    with tile.TileContext(nc) as tc:
        _build(nc, tc, params)
    nc.compile()
    return nc


def _build(nc, tc, params):
    ctxs = []

    def pool(name, bufs, space="SBUF"):
        p = tc.tile_pool(name=name, bufs=bufs, space=space)
        ctxs.append(p)
        return p.__enter__()

    const = pool("const", 1)
    pers = pool("pers", 1)
    big = pool("big", 65)          # [P,1+TL]-bf16-slab activation tiles
    scr = pool("scr", 9)           # [P,2TC]-slab scratch
    xsrc = pool("xsrc", 3)         # [P,1+TL]-f32 streamed sources
    sscr = pool("sscr", 10)        # small [P,S] scratch
    mtb = pool("mtb", 4)           # [P,TC]-bf16 per-head decay masks WD
    wts = pool("wts", 3)           # [128,2048]bf16 weight-blob ring
    psa = pool("psa", 3, space="PSUM")   # [P,2TC] f32 (2 banks)
    psb = pool("psb", 2, space="PSUM")   # [P,TC] f32 (1 bank)
    drm = pool("drm", 1, space="DRAM")

    cnt = [0]

    def bigt(dtype=bf16, cols=1 + TL):
        cnt[0] += 1
        return big.tile([P, cols], dtype, tag="big", name=f"b_{cnt[0]}")

    def sc(shape=(P, TC), dtype=f32):
        cnt[0] += 1
        return scr.tile(list(shape), dtype, tag="scr", name=f"sc_{cnt[0]}")

    def xsc():
        cnt[0] += 1
        return xsrc.tile([P, 1 + TL], f32, tag="xsrc", name=f"xs_{cnt[0]}")

    def ssc(shape=(P, S), dtype=f32):
        cnt[0] += 1
        return sscr.tile(list(shape), dtype, tag="sscr", name=f"ss_{cnt[0]}")

    def mt_tile():
        cnt[0] += 1
        return mtb.tile([P, TC], bf16, tag="mtb", name=f"mt_{cnt[0]}")

    def wt_tile(cols=2048):
        cnt[0] += 1
        return wts.tile([P, cols], bf16, tag="wt", name=f"wt_{cnt[0]}")

    def psa_():
        cnt[0] += 1
        return psa.tile([P, 2 * TC], f32, tag="psa", name=f"pa_{cnt[0]}")

    def pst_(shape=(P, TC), dtype=f32):
        cnt[0] += 1
        return psb.tile(list(shape), dtype, tag="psb", name=f"pb_{cnt[0]}")

    # x (bf16 pre-cast) DMAs issued first so they overlap const building
    XB = [bigt(cols=2 + TL) for _ in range(NK)]
    for k in range(NK):
        nc.sync.dma_start(XB[k][:], params["xTb"][k * P:(k + 1) * P, :])

    # ---------------- constants ----------------
    IOTA_T = const.tile([P, TC], f32, tag="iota_t")
    nc.gpsimd.iota(IOTA_T[:], pattern=[[1, TC]], base=0, channel_multiplier=0,
                   allow_small_or_imprecise_dtypes=True)
    IDENT = const.tile([P, P], bf16, tag="ident")
    make_identity(nc, IDENT[:])
    ONES_KB = const.tile([P, 1], bf16, tag="ones_kb")
    nc.gpsimd.memset(ONES_KB[:], 1.0)
    # GSEL[ch, c]: rows 0:64 set at col 30, rows 64:128 at col 31. Slicing
    # GSEL[:, 30-2p : 62-2p] yields a [128, 32] selector whose matmul
    # accumulates pair p's per-head column sums into rows 2p:2p+2.
    GSEL = const.tile([P, S - 2], bf16, tag="gsel")
    nc.gpsimd.memset(GSEL[:], 0.0)
    nc.gpsimd.memset(GSEL[0:S, 30:31], 1.0)
    nc.gpsimd.memset(GSEL[S:P, 31:32], 1.0)
    # IOTAW[p, j*64+c] = 511 - 128*j - p  (contrib decay exponents)
    IOTAW = const.tile([P, 4 * S], f32, tag="iotaw")
    nc.gpsimd.iota(IOTAW[:], pattern=[[-P, 4], [0, S]], base=TC - 1,
                   channel_multiplier=-1, allow_small_or_imprecise_dtypes=True)
    EPSB = const.tile([P, 1], f32, tag="epsb")
    nc.gpsimd.memset(EPSB[:], EPS)
    # IOTA_WD[p, x] = x - p - 1 where x > p else +1e30. exp(lnw * .) gives
    # the decay mask w^(x-1-p) as a function of x = t - jP, valid for every
    # 128-token block j (diag u term added separately on cols 0:128).
    IOTA_WD = const.tile([P, TC], f32, tag="iota_wd")
    iwd_raw = sc()
    nc.gpsimd.iota(iwd_raw[:], pattern=[[1, TC]], base=-1, channel_multiplier=-1,
                   allow_small_or_imprecise_dtypes=True)
    nc.gpsimd.affine_select(IOTA_WD[:], iwd_raw[:], pattern=[[1, TC]], base=-1,
                            channel_multiplier=-1, compare_op=AOT.is_ge,
                            fill=1e30)

    def ld(name, cols):
        t = const.tile([P, cols], f32, tag=name, name=name)
        nc.sync.dma_start(t[:], params[name][:])
        return t

    LN1G = ld("ln1g", NK); LN1B = ld("ln1b", NK)
    LN2G = ld("ln2g", NK); LN2B = ld("ln2b", NK)
    MXK = ld("mxk", NK); MXV = ld("mxv", NK); MXR = ld("mxr", NK)
    FMK = ld("fmk", NK); FMR = ld("fmr", NK)
    LNXG = ld("lnxg", NP); LNXB = ld("lnxb", NP)
    WSPP = ld("wspp", NP)
    LNWPP = ld("lnwpp", NP)

    def onem(src, name):
        t = const.tile([P, NK], f32, tag=name, name=name)
        nc.vector.tensor_scalar(t[:], src[:], -1.0, 1.0, AOT.mult, AOT.add)
        return t
    MXK1 = onem(MXK, "mxk1"); MXV1 = onem(MXV, "mxv1"); MXR1 = onem(MXR, "mxr1")
    FMK1 = onem(FMK, "fmk1"); FMR1 = onem(FMR, "fmr1")

    TD = const.tile([P, H], f32, tag="td")
    nc.sync.dma_start(TD[:], params["tdv"][0:1, :].partition_broadcast(P))
    UU = const.tile([P, H], f32, tag="uu")
    nc.sync.dma_start(UU[:], params["uv"][0:1, :].partition_broadcast(P))
    SMB = const.tile([P, 1], f32, tag="smb")
    nc.sync.dma_start(SMB[:], params["smask"][0:1, :].partition_broadcast(P))
    NEGLNW = const.tile([P, H], f32, tag="neglnw")
    nc.scalar.activation(NEGLNW[:], TD[:], AFT.Exp)
    LNW = const.tile([P, H], f32, tag="lnw")
    nc.vector.tensor_scalar_mul(LNW[:], NEGLNW[:], -1.0)

    xT = params["xT"]; yT = params["yT"]

    # DRAM tiles: collectives + x' spill + groupnorm broadcast bounce
    sout_d = drm.tile([P, NP * S], f32, tag="soutd")
    sgat_d = drm.tile([2 * P, NP * S], f32, tag="sgatd")
    xcol_d = drm.tile([P, NK], f32, tag="xcold")
    xcgat_d = drm.tile([2 * P, NK], f32, tag="xcgatd")
    xprime_d = drm.tile([C, TL], bf16, tag="xprd")
    rs_d = [drm.tile([H, TC], bf16, tag=f"rsd_{i}", name=f"rsd_{i}")
            for i in range(NCH)]
    mr_d = [drm.tile([H, TC], bf16, tag=f"mrd_{i}", name=f"mrd_{i}")
            for i in range(NCH)]

    # ---------- layernorm over channel dim (bf16, ts-pipelined) ----------
    def ln_pass(src_main, src_halo, g, b, dst_tiles, halo_mask,
                post_norm=None):
        """src_main(k, ts) -> [P, TC] bf16 aligned AP; src_halo(k) -> [P, 1]
        bf16 AP. Writes normalized bf16 into dst_tiles[k] ([P, 2+TL]: halo
        at col 1, main at cols 2:2+TL). ts=0 stats/chain/normalize issue
        before ts=1 stats so a ts-major GEMM can start on ts=0 columns
        while ts=1 normalizes. post_norm(k, ts) issues per-k mixes after
        each normalize. Ln/Exp batched (2 table loads per batch); the halo
        is chained with ts=1 so its AllGather (LN2) is covered."""
        pssA = psa_(); psqA = psa_()

        def stats(ts):
            for k in range(NK):
                s = src_main(k, ts)
                sq = sc((P, TC), bf16)
                nc.scalar.square(sq[:], s)
                nc.tensor.matmul(pssA[0:1, ts * TC:(ts + 1) * TC], ONES_KB[:],
                                 s, start=(k == 0), stop=(k == NK - 1))
                nc.tensor.matmul(psqA[0:1, ts * TC:(ts + 1) * TC], ONES_KB[:],
                                 sq[:], start=(k == 0), stop=(k == NK - 1))

        def chain_mv(pss, psq, n):
            m_ = sc((1, n)); nc.scalar.mul(m_[:], pss, 1.0 / C)
            q_ = sc((1, n)); nc.scalar.mul(q_[:], psq, 1.0 / C)
            msq = sc((1, n)); nc.vector.tensor_mul(msq[:], m_[:], m_[:])
            var = sc((1, n)); nc.vector.tensor_sub(var[:], q_[:], msq[:])
            return m_, var

        def chain_fin(items):
            # batched: all Ln, then all Exp (one table load each)
            lnvs = []
            for m_, var, n in items:
                lnv = sc((1, n))
                nc.scalar.activation(lnv[:], var[:], AFT.Ln,
                                     bias=EPSB[0:1, 0:1])
                lnvs.append(lnv)
            outs = []
            for (m_, var, n), lnv in zip(items, lnvs):
                rs = sc((1, n))
                nc.scalar.activation(rs[:], lnv[:], AFT.Exp, scale=-0.5)
                mrs = sc((1, n))
                nc.vector.scalar_tensor_tensor(mrs[:], m_[:], -1.0, rs[:],
                                               AOT.mult, AOT.mult)
                rsb = sc((1, n), bf16); nc.vector.tensor_copy(rsb[:], rs[:])
                mrb = sc((1, n), bf16); nc.vector.tensor_copy(mrb[:], mrs[:])
                brs = sc((P, n), bf16)
                nc.gpsimd.partition_broadcast(brs[:], rsb[:])
                bmrs = sc((P, n), bf16)
                nc.gpsimd.partition_broadcast(bmrs[:], mrb[:])
                outs.append((brs, bmrs))
            return outs

        def norm(k, ts, brs, bmrs):
            dst = dst_tiles[k]
            tmp = sc((P, TC), bf16)
            nc.vector.tensor_mul(tmp[:], src_main(k, ts), brs[:])
            nc.vector.tensor_add(tmp[:], tmp[:], bmrs[:])
            nc.vector.tensor_scalar(dst[:, 2 + ts * TC:2 + (ts + 1) * TC],
                                    tmp[:], g[:, k:k + 1], b[:, k:k + 1],
                                    AOT.mult, AOT.add)

        stats(0)
        m0, v0 = chain_mv(pssA[0:1, 0:TC], psqA[0:1, 0:TC], TC)
        (bc0,) = chain_fin([(m0, v0, TC)])
        stats(1)
        pssh = pst_((1, 1)); psqh = pst_((1, 1))
        for k in range(NK):
            hs = src_halo(k)
            hsq = sc((P, 1), bf16)
            nc.scalar.square(hsq[:], hs)
            nc.tensor.matmul(pssh[:], ONES_KB[:], hs,
                             start=(k == 0), stop=(k == NK - 1))
            nc.tensor.matmul(psqh[:], ONES_KB[:], hsq[:],
                             start=(k == 0), stop=(k == NK - 1))
        m1, v1 = chain_mv(pssA[0:1, TC:2 * TC], psqA[0:1, TC:2 * TC], TC)
        mh, vh = chain_mv(pssh[:], psqh[:], 1)
        bc1, bch = chain_fin([(m1, v1, TC), (mh, vh, 1)])
        # ts=0 normalize (no halo dependence)
        for k in range(NK):
            norm(k, 0, *bc0)
        # halo normalize (before the ts=0 mixes, which read col 1)
        for k in range(NK):
            dst = dst_tiles[k]
            tmp = sc((P, 1), bf16)
            nc.vector.tensor_mul(tmp[:], src_halo(k), bch[0][:])
            nc.vector.tensor_add(tmp[:], tmp[:], bch[1][:])
            nc.vector.tensor_scalar(dst[:, 1:2], tmp[:], g[:, k:k + 1],
                                    b[:, k:k + 1], AOT.mult, AOT.add)
            if halo_mask:
                nc.vector.tensor_scalar(dst[:, 1:2], dst[:, 1:2],
                                        SMB[:, 0:1], None, AOT.mult)
        if post_norm is not None:
            for k in range(NK):
                post_norm(k, 0)
        for k in range(NK):
            norm(k, 1, *bc1)
            if post_norm is not None:
                post_norm(k, 1)

    def mixf(dst, h, cf, k):
        """dst[:, 0:TL] = h_shift + cf[k]*(h - h_shift), full width."""
        dh = sc((P, TL), bf16)
        nc.vector.tensor_sub(dh[:], h[:, 2:2 + TL], h[:, 1:1 + TL])
        nc.vector.scalar_tensor_tensor(dst[:, 0:TL], dh[:], cf[:, k:k + 1],
                                       h[:, 1:1 + TL], AOT.mult, AOT.add)

    # ---------- GEMM helper: out[m] = sum_k w_tiled[.,m,k] ^T @ in[k] ----------
    def gemm_std(wt_dram, in_tiles, out_tiles, n_out, act=None, accum=False,
                 n_in=NK, G=8, col_base=0, post=None, pre=None,
                 ts_major=False):
        """Host-pretiled weights: blob cols ordered (mg, k, 256). m-groups
        of 2; per group ceil(n_in/G) blob DMAs of [128, G*256]; two 2-bank
        PSUM tiles (one per mi), double-buffered across groups.
        act: None -> copy; 'sigmoid'; 'sqrelu'. accum: add into out.
        post(m, pq): custom finalize; pre(mgl): prefetch hook."""
        nh = (n_in + G - 1) // G
        for mgl in range(n_out // 2):
            if pre is not None:
                pre(mgl)
            base = col_base + mgl * n_in * 2 * P
            wtl = []
            for hf in range(nh):
                kn = min(G, n_in - hf * G)
                w = wt_tile(kn * 2 * P)
                nc.sync.dma_start(
                    w[:], wt_dram[:, base + hf * G * 2 * P:
                                  base + (hf * G + kn) * 2 * P])
                wtl.append(w)
            pq = [psa_() for _ in range(2)]
            if ts_major:
                # all ts=0 matmuls before any ts=1: lets the GEMM start as
                # soon as ts=0 inputs exist (LN still normalizing ts=1)
                for ts in range(TS):
                    for k in range(n_in):
                        w = wtl[k // G]
                        co = (k % G) * 2 * P
                        for mi in range(2):
                            nc.tensor.matmul(
                                pq[mi][:, ts * TC:(ts + 1) * TC],
                                w[:, co + mi * P:co + (mi + 1) * P],
                                in_tiles[k][:, ts * TC:(ts + 1) * TC],
                                start=(k == 0), stop=(k == n_in - 1))
            else:
                for k in range(n_in):
                    w = wtl[k // G]
                    co = (k % G) * 2 * P
                    for mi in range(2):
                        for ts in range(TS):
                            nc.tensor.matmul(
                                pq[mi][:, ts * TC:(ts + 1) * TC],
                                w[:, co + mi * P:co + (mi + 1) * P],
                                in_tiles[k][:, ts * TC:(ts + 1) * TC],
                                start=(k == 0), stop=(k == n_in - 1))
            for mi in range(2):
                m = mgl * 2 + mi
                if post is not None:
                    post(m, pq[mi])
                    continue
                dst = out_tiles[m][:, 0:TL]
                src = pq[mi][0:P, 0:TL]
                if act == "sigmoid":
                    nc.scalar.activation(dst, src, AFT.Sigmoid)
                elif act == "sqrelu":
                    rl = xsc()
                    nc.scalar.activation(rl[:, 0:TL], src, AFT.Relu)
                    nc.vector.tensor_mul(dst, rl[:, 0:TL], rl[:, 0:TL])
                elif accum:
                    nc.vector.tensor_add(dst, dst, src)
                else:
                    nc.vector.tensor_copy(dst, src)

    # ================= phase 1: LN1 =================
    # x streamed once as a host-pre-cast bf16 copy (xTb); stats + normalize
    # both read it. K-mix issues inline per (k, ts) via post_norm.
    H1 = [bigt(cols=2 + TL) for _ in range(NK)]
    MQ = [bigt(cols=TL) for _ in range(NK)]

    def post1(k, ts):
        dh = sc((P, TC), bf16)
        nc.vector.tensor_sub(dh[:], H1[k][:, 2 + ts * TC:2 + (ts + 1) * TC],
                             H1[k][:, 1 + ts * TC:1 + (ts + 1) * TC])
        nc.vector.scalar_tensor_tensor(MQ[k][:, ts * TC:(ts + 1) * TC],
                                       dh[:], MXK[:, k:k + 1],
                                       H1[k][:, 1 + ts * TC:1 + (ts + 1) * TC],
                                       AOT.mult, AOT.add)

    ln_pass(lambda k, ts: XB[k][:, 2 + ts * TC:2 + (ts + 1) * TC],
            lambda k: XB[k][:, 1:2], LN1G, LN1B, H1, halo_mask=True,
            post_norm=post1)
    XB = None

    # ================= phase 2: K / V projections =================
    KT = [bigt(cols=TL) for _ in range(NK)]
    gemm_std(params["wk_t"], MQ, KT, NK)
    for k in range(NK):
        mixf(MQ[k], H1[k], MXV, k)
    # V transposed: VT[tslab (8 x 128 tokens)] as 2 tiles of [P, 1024] each.
    # wv_t blob cols ordered (cb, k, 512); stationary = MQ slab, moving = w.
    VT = [[bigt(cols=TL) for _ in range(2)] for _ in range(2 * 4)]
    for tg in range(2):       # t-slab groups of 4
        for cb in range(4):   # c_out banks of 512
            pv = [psa_() for _ in range(2)]
            for kq in range(4):
                w = wt_tile()
                nc.sync.dma_start(
                    w[:], params["wv_t"][:, (cb * NK + kq * 4) * TC:
                                         (cb * NK + kq * 4 + 4) * TC])
                for kk in range(4):
                    k = kq * 4 + kk
                    for ti in range(4):
                        tslab = tg * 4 + ti
                        nc.tensor.matmul(
                            pv[ti // 2][:, (ti % 2) * TC:(ti % 2 + 1) * TC],
                            MQ[k][:, tslab * P:(tslab + 1) * P],
                            w[:, kk * TC:(kk + 1) * TC],
                            start=(k == 0), stop=(k == NK - 1))
            for ti in range(4):
                nc.vector.tensor_copy(
                    VT[tg * 4 + ti][cb // 2][:, (cb % 2) * TC:(cb % 2 + 1) * TC],
                    pv[ti // 2][:, (ti % 2) * TC:(ti % 2 + 1) * TC])

    def vsl(i, j, h):
        """[P,S] value slice: chunk i, 128-token slab j, head h."""
        voff = h * S
        return VT[i * 4 + j][voff // TL][:, voff % TL:voff % TL + S]

    # ============ phase 3: state contributions + AllGather ============
    # K transposed per 128-token block with ONE full 128x128 transpose
    # covering both heads of the pair.
    CONTRIB0 = [pers.tile([P, S], f32, tag=f"c0_{p}", name=f"c0_{p}")
                for p in range(NP)]
    for p in range(NP):
        wkct = []
        for hh in range(2):
            h = 2 * p + hh
            t = sc((P, 4 * S), bf16)
            nc.scalar.activation(t[:], IOTAW[:], AFT.Exp,
                                 scale=LNW[:, h:h + 1])
            wkct.append(t)
        cts = []
        for i in range(NCH):
            ptrf = pst_((P, 4 * P), bf16)
            for j in range(4):
                nc.tensor.transpose(
                    ptrf[:, j * P:(j + 1) * P],
                    KT[p][:, i * TC + j * P:i * TC + (j + 1) * P],
                    IDENT[:])
            kw = sc((P, 4 * P), bf16)
            for j in range(4):
                for hh in range(2):
                    o = j * P + hh * S
                    nc.vector.tensor_mul(kw[:, o:o + S], ptrf[:, o:o + S],
                                         wkct[hh][:, j * S:(j + 1) * S])
            pst = pst_((P, S))
            for hh in range(2):
                h = 2 * p + hh
                pr = slice(hh * S, hh * S + S)
                for j in range(4):
                    nc.tensor.matmul(pst[pr, :], kw[:, j * P + hh * S:
                                                    j * P + hh * S + S],
                                     vsl(i, j, h),
                                     start=(j == 0), stop=(j == 3))
            if i == 0:
                nc.vector.tensor_copy(CONTRIB0[p][:], pst[:])
                cts.append(CONTRIB0[p])
            else:
                c1 = ssc()
                nc.vector.tensor_copy(c1[:], pst[:])
                cts.append(c1)
        so = ssc()
        nc.vector.scalar_tensor_tensor(so[:], cts[0][:], WSPP[:, p:p + 1],
                                       cts[1][:], AOT.mult, AOT.add)
        nc.sync.dma_start(sout_d[:, p * S:(p + 1) * S], so[:])
    nc.gpsimd.collective_compute("AllGather", AOT.bypass, replica_groups=GROUPS,
                                 ins=[sout_d.opt()], outs=[sgat_d.opt()])

    # ================= phase 4: R projection (overlaps AG) =================
    for k in range(NK):
        mixf(MQ[k], H1[k], MXR, k)
    H1 = None
    RT = [bigt(cols=TL) for _ in range(NK)]
    gemm_std(params["wr_t"], MQ, RT, NK)
    MQ = None

    # incoming state = smask * (rank0 shard of gather)
    SIN = pers.tile([P, NP * S], f32, tag="sin", name="sin")
    nc.sync.dma_start(SIN[:], sgat_d[0:P, :])
    nc.vector.tensor_scalar(SIN[:], SIN[:], SMB[:, 0:1], None, AOT.mult)

    # ================= phase 5: attention =================
    # Decay mask per head = column-shifted view of M_h[p,c]=w^(c-p) (c>=p)
    # plus diagonal-block D_h[p,q]=w^(q-p-1) (q>p) + u_h*I. Mask is zero
    # for t <= jP (except diag), so pa2/pout matmuls are restricted to
    # cols >= jP. Groupnorm stats are collected per (pair, chunk) into a
    # batched [64, TC] tile; one scalar chain per chunk; normalization
    # applied with gpsimd partition-broadcasts (no tensor-engine work).
    XA = [bigt(cols=TL) for _ in range(NK)]
    PSALL = {}

    def attn_pair(i, p, masks=None):
        h0, h1 = 2 * p, 2 * p + 1
        if masks is None:
            masks = build_masks(p)
        WD_ = masks
        wb = sc(dtype=bf16)
        nc.scalar.activation(wb[:], IOTA_T[:], AFT.Exp,
                             scale=LNWPP[:, p:p + 1])
        st_mm = ssc((P, S), bf16)
        if i == 0:
            nc.vector.tensor_copy(st_mm[:], SIN[:, p * S:(p + 1) * S])
        else:
            stt = ssc()
            nc.vector.scalar_tensor_tensor(stt[:], SIN[:, p * S:(p + 1) * S],
                                           WSPP[:, p:p + 1], CONTRIB0[p][:],
                                           AOT.mult, AOT.add)
            nc.vector.tensor_copy(st_mm[:], stt[:])
        rtw = sc(dtype=bf16)
        nc.vector.tensor_mul(rtw[:], RT[p][:, i * TC:(i + 1) * TC], wb[:])
        pout = pst_((P, TC))

        def pout_mms(j, ast):
            for hh in range(2):
                h = 2 * p + hh
                pr = slice(hh * S, hh * S + S)
                nc.tensor.matmul(pout[pr, j * P:TC], vsl(i, j, h),
                                 ast[:, hh * TC + j * P:(hh + 1) * TC],
                                 start=False, stop=(j == 3))

        prev = None
        for j in range(4):
            pa2 = psa_()
            for hh in range(2):
                pr = slice(hh * S, hh * S + S)
                nc.tensor.matmul(
                    pa2[:, hh * TC + j * P:(hh + 1) * TC],
                    KT[p][pr, i * TC + j * P:i * TC + (j + 1) * P],
                    RT[p][pr, i * TC + j * P:(i + 1) * TC],
                    start=True, stop=True)
            if j == 0:
                # state term (start=True clears the pout bank); issued
                # after the first pa2 so the tensor queue isn't blocked
                # waiting on wb/rtw from the scalar/vector engines.
                for hh in range(2):
                    pr = slice(hh * S, hh * S + S)
                    nc.tensor.matmul(pout[pr, :], st_mm[pr, :], rtw[pr, :],
                                     start=True, stop=False)
            ast = sc((P, 2 * TC), bf16)
            for hh in range(2):
                o = hh * TC + j * P
                nc.vector.tensor_mul(ast[:, o:(hh + 1) * TC],
                                     pa2[:, o:(hh + 1) * TC],
                                     WD_[hh][:, 0:TC - j * P])
            if prev is not None:
                pout_mms(*prev)
            prev = (j, ast)
        pout_mms(*prev)
        # raw attention out + gathered groupnorm stats (GSEL accumulate);
        # psum copy + square on the scalar engine (vector is the pacer here)
        nc.scalar.copy(XA[p][:, i * TC:(i + 1) * TC], pout[:])
        sq = sc((P, TC), bf16)
        nc.scalar.square(sq[:], XA[p][:, i * TC:(i + 1) * TC])
        if p == 0:
            PSALL[i] = psa_()
        sel = GSEL[:, 30 - 2 * p:62 - 2 * p]
        nc.tensor.matmul(PSALL[i][0:H, 0:TC], sel,
                         XA[p][:, i * TC:(i + 1) * TC],
                         start=(p == 0), stop=(p == NP - 1))
        nc.tensor.matmul(PSALL[i][0:H, TC:2 * TC], sel, sq[:],
                         start=(p == 0), stop=(p == NP - 1))

    def build_masks(p):
        WD_ = []
        for hh in range(2):
            h = 2 * p + hh
            wd = mt_tile()
            nc.scalar.activation(wd[:], IOTA_WD[:], AFT.Exp,
                                 scale=LNW[:, h:h + 1])
            # diagonal 'u' bonus lands in the first 128-col block
            nc.vector.scalar_tensor_tensor(wd[:, 0:P], IDENT[:],
                                           UU[:, h:h + 1], wd[:, 0:P],
                                           AOT.mult, AOT.add)
            WD_.append(wd)
        return WD_

    def gn_chain(i):
        ps = PSALL.pop(i)
        m_ = sc((H, TC))
        nc.scalar.mul(m_[:], ps[0:H, 0:TC], 1.0 / (S * HS_DIV))
        q_ = sc((H, TC))
        nc.scalar.mul(q_[:], ps[0:H, TC:2 * TC],
                      1.0 / (S * HS_DIV * HS_DIV))
        msq = sc((H, TC)); nc.vector.tensor_mul(msq[:], m_[:], m_[:])
        var = sc((H, TC)); nc.vector.tensor_sub(var[:], q_[:], msq[:])
        lnv = sc((H, TC))
        nc.scalar.activation(lnv[:], var[:], AFT.Ln, bias=EPSB[0:H, 0:1])
        rs = sc((H, TC))
        nc.scalar.activation(rs[:], lnv[:], AFT.Exp, scale=-0.5)
        mrs = sc((H, TC))
        nc.vector.scalar_tensor_tensor(mrs[:], m_[:], -1.0, rs[:],
                                       AOT.mult, AOT.mult)
        rsh = sc((H, TC))
        nc.vector.tensor_scalar_mul(rsh[:], rs[:], 1.0 / HS_DIV)
        rsb = sc((H, TC), bf16)
        nc.vector.tensor_copy(rsb[:], rsh[:])
        mrb = sc((H, TC), bf16)
        nc.vector.tensor_copy(mrb[:], mrs[:])
        nc.sync.dma_start(rs_d[i][:], rsb[:])
        nc.sync.dma_start(mr_d[i][:], mrb[:])

    def gn_finish(i, p):
        # per-head broadcast of the groupnorm scale/shift rows via a DMA
        # bounce through DRAM (engines can't read non-32-aligned rows).
        brs = sc(dtype=bf16)
        bmrs = sc(dtype=bf16)
        for hh in range(2):
            r = 2 * p + hh
            pr = slice(hh * S, hh * S + S)
            nc.sync.dma_start(brs[pr, :],
                              rs_d[i][r:r + 1, :].partition_broadcast(S))
            nc.sync.dma_start(bmrs[pr, :],
                              mr_d[i][r:r + 1, :].partition_broadcast(S))
        xa = sc(dtype=bf16)
        nc.vector.tensor_mul(xa[:], XA[p][:, i * TC:(i + 1) * TC], brs[:])
        nc.vector.tensor_add(xa[:], xa[:], bmrs[:])
        nc.vector.tensor_scalar(XA[p][:, i * TC:(i + 1) * TC], xa[:],
                                LNXG[:, p:p + 1], LNXB[:, p:p + 1],
                                AOT.mult, AOT.add)

    for p in range(NP):
        attn_pair(0, p)
    mk0 = build_masks(0)
    mk1 = build_masks(1)
    attn_pair(1, 0, mk0)
    attn_pair(1, 1, mk1)
    gn_chain(0)
    for p in range(2, NP):
        attn_pair(1, p)
        gn_finish(0, p - 2)
    gn_finish(0, NP - 2)
    gn_finish(0, NP - 1)
    gn_chain(1)
    for p in range(NP):
        gn_finish(1, p)
    RT = KT = VT = None

    # ================= phase 6: Wo + residual, spill x' =================
    XP = [bigt(cols=TL) for _ in range(NK)]
    xr_tiles = {}

    def wo_pre(mgl):
        for m in (2 * mgl, 2 * mgl + 1):
            t = xsc()
            nc.sync.dma_start(t[:], xT[m * P:(m + 1) * P, :])
            xr_tiles[m] = t

    def wo_post(m, pq):
        xr = xr_tiles.pop(m)
        for ts in range(TS):
            c0, c1 = ts * TC, (ts + 1) * TC
            nc.vector.tensor_add(XP[m][:, c0:c1], pq[0:P, c0:c1],
                                 xr[:, 1 + c0:1 + c1])
            nc.sync.dma_start(xprime_d[m * P:(m + 1) * P, c0:c1],
                              XP[m][:, c0:c1])
        lc = sc((P, 1))
        nc.vector.tensor_copy(lc[:], XP[m][:, TL - 1:TL])
        nc.sync.dma_start(xcol_d[:, m:m + 1], lc[:])

    gemm_std(params["wo_t"], XA, XP, NK, post=wo_post, pre=wo_pre)
    XA = None
    nc.gpsimd.collective_compute("AllGather", AOT.bypass, replica_groups=GROUPS,
                                 ins=[xcol_d.opt()], outs=[xcgat_d.opt()])

    # ================= phase 7: LN2 + mixes =================
    XCH = const.tile([P, NK], f32, tag="xch")   # per-chunk halo cols
    nc.sync.dma_start(XCH[:], xcgat_d[0:P, :])
    XCHB = const.tile([P, NK], bf16, tag="xchb")
    nc.vector.tensor_copy(XCHB[:], XCH[:])

    H2 = [bigt(cols=2 + TL) for _ in range(NK)]
    MFK = [bigt(cols=TL) for _ in range(NK)]
    MFR = [bigt(cols=TL) for _ in range(NK)]

    def post2(k, ts):
        dh = sc((P, TC), bf16)
        nc.vector.tensor_sub(dh[:], H2[k][:, 2 + ts * TC:2 + (ts + 1) * TC],
                             H2[k][:, 1 + ts * TC:1 + (ts + 1) * TC])
        hsv = H2[k][:, 1 + ts * TC:1 + (ts + 1) * TC]
        nc.vector.scalar_tensor_tensor(MFK[k][:, ts * TC:(ts + 1) * TC],
                                       dh[:], FMK[:, k:k + 1], hsv,
                                       AOT.mult, AOT.add)
        nc.vector.scalar_tensor_tensor(MFR[k][:, ts * TC:(ts + 1) * TC],
                                       dh[:], FMR[:, k:k + 1], hsv,
                                       AOT.mult, AOT.add)

    ln_pass(lambda k, ts: XP[k][:, ts * TC:(ts + 1) * TC],
            lambda k: XCHB[:, k:k + 1], LN2G, LN2B, H2, halo_mask=True,
            post_norm=post2)
    XP = None
    H2 = None

    # ================= phase 8: gate = sigmoid(mfr @ wfr) =================
    GT = [bigt(cols=TL) for _ in range(NK)]
    gemm_std(params["wfr_t"], MFR, GT, NK, act="sigmoid")
    MFR = None

    # ========== phase 9: FFN quarters: kf=relu^2(mfk@wfk); kv+=wfv^T@kf ==========
    # last quarter's Wfv finalize writes y = x' + gate*kv directly.
    KV = [bigt(cols=TL) for _ in range(NK)]
    xp_tiles = {}

    def y_pre(mgl):
        for m in (2 * mgl, 2 * mgl + 1):
            t = sc((P, TL), bf16)
            nc.sync.dma_start(t[:], xprime_d[m * P:(m + 1) * P, :])
            xp_tiles[m] = t

    def y_post(m, pq):
        xp = xp_tiles.pop(m)
        for ts in range(TS):
            c0, c1 = ts * TC, (ts + 1) * TC
            kvf = sc()
            nc.vector.tensor_add(kvf[:], KV[m][:, c0:c1], pq[0:P, c0:c1])
            gk = sc()
            nc.vector.tensor_mul(gk[:], GT[m][:, c0:c1], kvf[:])
            yo = sc()
            nc.vector.tensor_add(yo[:], xp[:, c0:c1], gk[:])
            nc.sync.dma_start(yT[m * P:(m + 1) * P, c0:c1], yo[:])

    for q in range(NQ):
        KF = [bigt(cols=TL) for _ in range(JQ)]
        gemm_std(params["wfk_t"], MFK, KF, JQ, act="sqrelu",
                 col_base=q * (JQ // 2) * NK * 2 * P)
        # kv partial: contract the quarter's 14 j-chunks
        if q < NQ - 1:
            gemm_std(params["wfv_t"], KF, KV, NK, accum=(q > 0), n_in=JQ, G=7,
                     col_base=q * (NK // 2) * JQ * 2 * P)
        else:
            gemm_std(params["wfv_t"], KF, KV, NK, n_in=JQ, G=7,
                     col_base=q * (NK // 2) * JQ * 2 * P,
                     post=y_post, pre=y_pre)
        KF = None
    MFK = None

    for c in reversed(ctxs):
        c.__exit__(None, None, None)


# ----------------------------------------------------------------------
# Host-side sharding / gather
# ----------------------------------------------------------------------
import ml_dtypes

_NC_CACHE = {}


def _vec_pk(v, nk=NK):
    return np.ascontiguousarray(np.asarray(v).reshape(nk, P).T.astype(np.float32))


def _make_in_maps(inputs):
    x = np.asarray(inputs["x"], np.float32)
    bf = ml_dtypes.bfloat16
    td = np.asarray(inputs["time_decay"], np.float64)
    w = np.exp(-np.exp(td))                      # [H]
    ws = w ** TC
    wspp = np.zeros((P, NP), np.float32)
    lnwpp = np.zeros((P, NP), np.float32)
    lnw = -np.exp(td)
    for p in range(NP):
        wspp[0:S, p] = ws[2 * p]
        wspp[S:P, p] = ws[2 * p + 1]
        lnwpp[0:S, p] = lnw[2 * p]
        lnwpp[S:P, p] = lnw[2 * p + 1]
    def _tile_mk(W):
        """[n_in*128, n_mg*256] -> [128, n_mg*n_in*256], cols (mg, k, c)."""
        n_in = W.shape[0] // P
        n_mg = W.shape[1] // (2 * P)
        return np.ascontiguousarray(
            W.reshape(n_in, P, n_mg, 2 * P).transpose(1, 2, 0, 3)
             .reshape(P, -1).astype(bf))

    wcache = {}
    for nm, key in [("wr_t", "Wr"), ("wk_t", "Wk"), ("wo_t", "Wo"),
                    ("wfr_t", "Wfr"), ("wfk_t", "Wfk")]:
        wcache[nm] = _tile_mk(np.asarray(inputs[key], np.float32))
    Wfv = np.asarray(inputs["Wfv"], np.float32)
    wcache["wfv_t"] = np.ascontiguousarray(np.concatenate(
        [_tile_mk(Wfv[q * JQ * P:(q + 1) * JQ * P, :]) for q in range(NQ)],
        axis=1))
    Wv = np.asarray(inputs["Wv"], np.float32)
    wcache["wv_t"] = np.ascontiguousarray(
        Wv.reshape(NK, P, 4, TC).transpose(1, 2, 0, 3).reshape(P, -1).astype(bf))
    maps = []
    for c in range(8):
        b, half = c // 2, c % 2
        t0 = half * TL
        xh = np.zeros((C, 1 + TL), np.float32)
        xh[:, 1:] = x[b, t0:t0 + TL, :].T
        if half == 1:
            xh[:, 0] = x[b, t0 - 1, :]
        xhb = np.zeros((C, 2 + TL), np.float32)
        xhb[:, 1:] = xh
        maps.append({
            "xT": np.ascontiguousarray(xh),
            "xTb": np.ascontiguousarray(xhb.astype(bf)),
            **wcache,
            "wspp": wspp, "lnwpp": lnwpp,
            "smask": np.full((1, 1), float(half), np.float32),
            "ln1g": _vec_pk(inputs["ln1_g"]), "ln1b": _vec_pk(inputs["ln1_b"]),
            "ln2g": _vec_pk(inputs["ln2_g"]), "ln2b": _vec_pk(inputs["ln2_b"]),
            "mxk": _vec_pk(inputs["att_mix_k"]), "mxv": _vec_pk(inputs["att_mix_v"]),
            "mxr": _vec_pk(inputs["att_mix_r"]),
            "fmk": _vec_pk(inputs["ffn_mix_k"]), "fmr": _vec_pk(inputs["ffn_mix_r"]),
            "lnxg": _vec_pk(inputs["lnx_g"], NP),
            "lnxb": _vec_pk(inputs["lnx_b"], NP),
            "tdv": np.ascontiguousarray(np.asarray(inputs["time_decay"],
                                                   np.float32)[None, :]),
            "uv": np.ascontiguousarray(np.asarray(inputs["time_faaaa"],
                                                  np.float32)[None, :]),
        })
    return maps


def run_on_hw(inputs, trace=False):
    from concourse.bass_utils import run_bass_kernel_spmd
    if "nc" not in _NC_CACHE:
        _NC_CACHE["nc"] = build_nc()
    nc = _NC_CACHE["nc"]
    maps = _make_in_maps(inputs)
    res = run_bass_kernel_spmd(nc, maps, core_ids=list(range(8)), trace=trace)
    B = 4
    out = np.zeros((B, 2 * TL, C), np.float32)
    for c in range(8):
        b, half = c // 2, c % 2
        out[b, half * TL:(half + 1) * TL, :] = res.results[c]["yT"].T
    return out, res


def kernel(**inputs) -> np.ndarray:
    out, _ = run_on_hw(inputs, trace=False)
    return out


# revision 34
# speedup vs baseline: 1.0444x; 1.0150x over previous
"""RWKV5 block, sequence-parallel across 8 trn2 cores.

Core c -> batch c//2, sequence half c%2 (tokens t0 = half*1024, TL=1024
= 2 recurrence chunks of TC=512). Each core runs FULL-width GEMMs
(C=2048, DF=7168) on its token half; every weight is streamed from HBM
once (Wv twice). Cross-core traffic per pair: one 512KB state AllGather
(recurrent state after chunk 1 -> second half) plus an 8KB x' halo
column AllGather for the ChannelMix time-shift.

Layout: activations channel-major [C, T]. v kept time-major [T, C]
(VT) for the attention a@v and k^T@v contractions.

v2: scheduling-focused rewrite. Attention decay masks are built as
column-shifted views of one per-head exp table (M) plus a diagonal
block tile (D), pa2/pout matmuls are column-restricted to the nonzero
mask region, groupnorm is deferred into a batched per-chunk pass with
gpsimd partition-broadcasts (no fp32 matmuls, no per-pair table
swaps), LN stats run on bf16 operands, and the final FFN quarter
writes y directly.
"""
import numpy as np
import concourse.bass as bass
import concourse.mybir as mybir
import concourse.tile as tile
from concourse import bacc
from concourse.masks import make_identity

f32 = mybir.dt.float32
bf16 = mybir.dt.bfloat16
AOT = mybir.AluOpType
AFT = mybir.ActivationFunctionType

C = 2048
H = 32         # heads
S = 64         # head dim
TC = 512       # recurrence chunk
TL = 1024      # local tokens per core
NCH = TL // TC # 2 local chunks
DF = 7168
P = 128
NK = C // P    # 16 channel chunks
NP = H // 2    # 16 head pairs
NJ = DF // P   # 56
NQ = 4         # DF quarters
JQ = NJ // NQ  # 14 j-chunks per quarter
EPS = 1e-5
HS_DIV = float(np.sqrt(S))
GROUPS = [[0, 1], [2, 3], [4, 5], [6, 7]]
TS = TL // TC  # 2 column sub-ranges of 512


def build_nc():
    nc = bacc.Bacc("TRN2", target_bir_lowering=False, debug=False, num_devices=8)
    dp = nc.declare_dram_parameter
    params = {
        "xT": dp("xT", [C, 1 + TL], f32, isOutput=False),
        # bf16 pre-cast copy for LN1: col 0 unused, col 1 halo, cols 2:2+TL
        # data (so 512-col slices start 4B-aligned for DVE 2x mode)
        "xTb": dp("xTb", [C, 2 + TL], bf16, isOutput=False),
        # weights pre-tiled on host: cols ordered (m-group, k, col-in-tile)
        "wr_t": dp("wr_t", [P, C * C // P], bf16, isOutput=False),
        "wk_t": dp("wk_t", [P, C * C // P], bf16, isOutput=False),
        "wv_t": dp("wv_t", [P, C * C // P], bf16, isOutput=False),
        "wo_t": dp("wo_t", [P, C * C // P], bf16, isOutput=False),
        "wfk_t": dp("wfk_t", [P, C * DF // P], bf16, isOutput=False),
        "wfv_t": dp("wfv_t", [P, C * DF // P], bf16, isOutput=False),
        "wfr_t": dp("wfr_t", [P, C * C // P], bf16, isOutput=False),
        "wspp": dp("wspp", [P, NP], f32, isOutput=False),
        "lnwpp": dp("lnwpp", [P, NP], f32, isOutput=False),
        "smask": dp("smask", [1, 1], f32, isOutput=False),
        "tdv": dp("tdv", [1, H], f32, isOutput=False),
        "uv": dp("uv", [1, H], f32, isOutput=False),
        "yT": dp("yT", [C, TL], f32, isOutput=True),
    }
    for nm, cols in [("ln1g", NK), ("ln1b", NK), ("ln2g", NK), ("ln2b", NK),
                     ("mxk", NK), ("mxv", NK), ("mxr", NK), ("fmk", NK),
                     ("fmr", NK), ("lnxg", NP), ("lnxb", NP)]:
        params[nm] = dp(nm, [P, cols], f32, isOutput=False)
    with tile.TileContext(nc) as tc:
        _build(nc, tc, params)
    nc.compile()
    return nc


def _build(nc, tc, params):
    ctxs = []

    def pool(name, bufs, space="SBUF"):
        p = tc.tile_pool(name=name, bufs=bufs, space=space)
        ctxs.append(p)
        return p.__enter__()

    const = pool("const", 1)
    pers = pool("pers", 1)
    big = pool("big", 65)          # [P,1+TL]-bf16-slab activation tiles
    scr = pool("scr", 9)           # [P,2TC]-slab scratch
    xsrc = pool("xsrc", 3)         # [P,1+TL]-f32 streamed sources
    sscr = pool("sscr", 10)        # small [P,S] scratch
    mtb = pool("mtb", 4)           # [P,TC]-bf16 per-head decay masks WD
    wts = pool("wts", 3)           # [128,2048]bf16 weight-blob ring
    psa = pool("psa", 3, space="PSUM")   # [P,2TC] f32 (2 banks)
    psb = pool("psb", 2, space="PSUM")   # [P,TC] f32 (1 bank)
    drm = pool("drm", 1, space="DRAM")

    cnt = [0]

    def bigt(dtype=bf16, cols=1 + TL):
        cnt[0] += 1
        return big.tile([P, cols], dtype, tag="big", name=f"b_{cnt[0]}")

    def sc(shape=(P, TC), dtype=f32):
        cnt[0] += 1
        return scr.tile(list(shape), dtype, tag="scr", name=f"sc_{cnt[0]}")

    def xsc():
        cnt[0] += 1
        return xsrc.tile([P, 1 + TL], f32, tag="xsrc", name=f"xs_{cnt[0]}")

    def ssc(shape=(P, S), dtype=f32):
        cnt[0] += 1
        return sscr.tile(list(shape), dtype, tag="sscr", name=f"ss_{cnt[0]}")

    def mt_tile():
        cnt[0] += 1
        return mtb.tile([P, 2 * TC], bf16, tag="mtb", name=f"mt_{cnt[0]}")

    def wt_tile(cols=2048):
        cnt[0] += 1
        return wts.tile([P, cols], bf16, tag="wt", name=f"wt_{cnt[0]}")

    def psa_():
        cnt[0] += 1
        return psa.tile([P, 2 * TC], f32, tag="psa", name=f"pa_{cnt[0]}")

    def pst_(shape=(P, TC), dtype=f32):
        cnt[0] += 1
        return psb.tile(list(shape), dtype, tag="psb", name=f"pb_{cnt[0]}")

    # x (bf16 pre-cast) DMAs issued first so they overlap const building
    XB = [bigt(cols=2 + TL) for _ in range(NK)]
    for k in range(NK):
        nc.sync.dma_start(XB[k][:], params["xTb"][k * P:(k + 1) * P, :])

    # ---------------- constants ----------------
    IOTA_T = const.tile([P, TC], f32, tag="iota_t")
    nc.gpsimd.iota(IOTA_T[:], pattern=[[1, TC]], base=0, channel_multiplier=0,
                   allow_small_or_imprecise_dtypes=True)
    IDENT = const.tile([P, P], bf16, tag="ident")
    make_identity(nc, IDENT[:])
    ONES_KB = const.tile([P, 1], bf16, tag="ones_kb")
    nc.gpsimd.memset(ONES_KB[:], 1.0)
    # GSEL[ch, c]: rows 0:64 set at col 30, rows 64:128 at col 31. Slicing
    # GSEL[:, 30-2p : 62-2p] yields a [128, 32] selector whose matmul
    # accumulates pair p's per-head column sums into rows 2p:2p+2.
    GSEL = const.tile([P, S - 2], bf16, tag="gsel")
    nc.gpsimd.memset(GSEL[:], 0.0)
    nc.gpsimd.memset(GSEL[0:S, 30:31], 1.0)
    nc.gpsimd.memset(GSEL[S:P, 31:32], 1.0)
    # IOTAW[p, j*64+c] = 511 - 128*j - p  (contrib decay exponents)
    IOTAW = const.tile([P, 4 * S], f32, tag="iotaw")
    nc.gpsimd.iota(IOTAW[:], pattern=[[-P, 4], [0, S]], base=TC - 1,
                   channel_multiplier=-1, allow_small_or_imprecise_dtypes=True)
    EPSB = const.tile([P, 1], f32, tag="epsb")
    nc.gpsimd.memset(EPSB[:], EPS)
    # IOTA_WD[p, x] = x - p - 1 where x > p else +1e30. exp(lnw * .) gives
    # the decay mask w^(x-1-p) as a function of x = t - jP, valid for every
    # 128-token block j (diag u term added separately on cols 0:128).
    IOTA_WD = const.tile([P, TC], f32, tag="iota_wd")
    iwd_raw = sc()
    nc.gpsimd.iota(iwd_raw[:], pattern=[[1, TC]], base=-1, channel_multiplier=-1,
                   allow_small_or_imprecise_dtypes=True)
    nc.gpsimd.affine_select(IOTA_WD[:], iwd_raw[:], pattern=[[1, TC]], base=-1,
                            channel_multiplier=-1, compare_op=AOT.is_ge,
                            fill=1e30)

    def ld(name, cols):
        t = const.tile([P, cols], f32, tag=name, name=name)
        nc.sync.dma_start(t[:], params[name][:])
        return t

    LN1G = ld("ln1g", NK); LN1B = ld("ln1b", NK)
    LN2G = ld("ln2g", NK); LN2B = ld("ln2b", NK)
    MXK = ld("mxk", NK); MXV = ld("mxv", NK); MXR = ld("mxr", NK)
    FMK = ld("fmk", NK); FMR = ld("fmr", NK)
    LNXG = ld("lnxg", NP); LNXB = ld("lnxb", NP)
    WSPP = ld("wspp", NP)
    LNWPP = ld("lnwpp", NP)

    def onem(src, name):
        t = const.tile([P, NK], f32, tag=name, name=name)
        nc.vector.tensor_scalar(t[:], src[:], -1.0, 1.0, AOT.mult, AOT.add)
        return t
    MXK1 = onem(MXK, "mxk1"); MXV1 = onem(MXV, "mxv1"); MXR1 = onem(MXR, "mxr1")
    FMK1 = onem(FMK, "fmk1"); FMR1 = onem(FMR, "fmr1")

    TD = const.tile([P, H], f32, tag="td")
    nc.sync.dma_start(TD[:], params["tdv"][0:1, :].partition_broadcast(P))
    UU = const.tile([P, H], f32, tag="uu")
    nc.sync.dma_start(UU[:], params["uv"][0:1, :].partition_broadcast(P))
    SMB = const.tile([P, 1], f32, tag="smb")
    nc.sync.dma_start(SMB[:], params["smask"][0:1, :].partition_broadcast(P))
    NEGLNW = const.tile([P, H], f32, tag="neglnw")
    nc.scalar.activation(NEGLNW[:], TD[:], AFT.Exp)
    LNW = const.tile([P, H], f32, tag="lnw")
    nc.vector.tensor_scalar_mul(LNW[:], NEGLNW[:], -1.0)

    xT = params["xT"]; yT = params["yT"]

    # DRAM tiles: collectives + x' spill + groupnorm broadcast bounce
    sout_d = drm.tile([P, NP * S], f32, tag="soutd")
    sgat_d = drm.tile([2 * P, NP * S], f32, tag="sgatd")
    xcol_d = drm.tile([P, NK], f32, tag="xcold")
    xcgat_d = drm.tile([2 * P, NK], f32, tag="xcgatd")
    xprime_d = drm.tile([C, TL], bf16, tag="xprd")
    rs_d = [drm.tile([H, TC], bf16, tag=f"rsd_{i}", name=f"rsd_{i}")
            for i in range(NCH)]
    mr_d = [drm.tile([H, TC], bf16, tag=f"mrd_{i}", name=f"mrd_{i}")
            for i in range(NCH)]

    # ---------- layernorm over channel dim (bf16, ts-pipelined) ----------
    def ln_pass(src_main, src_halo, g, b, dst_tiles, halo_mask,
                post_norm=None):
        """src_main(k, ts) -> [P, TC] bf16 aligned AP; src_halo(k) -> [P, 1]
        bf16 AP. Writes normalized bf16 into dst_tiles[k] ([P, 2+TL]: halo
        at col 1, main at cols 2:2+TL). ts=0 stats/chain/normalize issue
        before ts=1 stats so a ts-major GEMM can start on ts=0 columns
        while ts=1 normalizes. post_norm(k, ts) issues per-k mixes after
        each normalize. Ln/Exp batched (2 table loads per batch); the halo
        is chained with ts=1 so its AllGather (LN2) is covered."""
        pssA = psa_(); psqA = psa_()

        def stats(ts):
            for k in range(NK):
                s = src_main(k, ts)
                sq = sc((P, TC), bf16)
                nc.scalar.square(sq[:], s)
                nc.tensor.matmul(pssA[0:1, ts * TC:(ts + 1) * TC], ONES_KB[:],
                                 s, start=(k == 0), stop=(k == NK - 1))
                nc.tensor.matmul(psqA[0:1, ts * TC:(ts + 1) * TC], ONES_KB[:],
                                 sq[:], start=(k == 0), stop=(k == NK - 1))

        def chain_mv(pss, psq, n):
            m_ = sc((1, n)); nc.scalar.mul(m_[:], pss, 1.0 / C)
            q_ = sc((1, n)); nc.scalar.mul(q_[:], psq, 1.0 / C)
            msq = sc((1, n)); nc.vector.tensor_mul(msq[:], m_[:], m_[:])
            var = sc((1, n)); nc.vector.tensor_sub(var[:], q_[:], msq[:])
            return m_, var

        def chain_fin(items):
            # batched: all Ln, then all Exp (one table load each)
            lnvs = []
            for m_, var, n in items:
                lnv = sc((1, n))
                nc.scalar.activation(lnv[:], var[:], AFT.Ln,
                                     bias=EPSB[0:1, 0:1])
                lnvs.append(lnv)
            outs = []
            for (m_, var, n), lnv in zip(items, lnvs):
                rs = sc((1, n))
                nc.scalar.activation(rs[:], lnv[:], AFT.Exp, scale=-0.5)
                mrs = sc((1, n))
                nc.vector.scalar_tensor_tensor(mrs[:], m_[:], -1.0, rs[:],
                                               AOT.mult, AOT.mult)
                rsb = sc((1, n), bf16); nc.vector.tensor_copy(rsb[:], rs[:])
                mrb = sc((1, n), bf16); nc.vector.tensor_copy(mrb[:], mrs[:])
                brs = sc((P, n), bf16)
                nc.gpsimd.partition_broadcast(brs[:], rsb[:])
                bmrs = sc((P, n), bf16)
                nc.gpsimd.partition_broadcast(bmrs[:], mrb[:])
                outs.append((brs, bmrs))
            return outs

        def norm(k, ts, brs, bmrs):
            dst = dst_tiles[k]
            tmp = sc((P, TC), bf16)
            nc.vector.tensor_mul(tmp[:], src_main(k, ts), brs[:])
            nc.vector.tensor_add(tmp[:], tmp[:], bmrs[:])
            nc.vector.tensor_scalar(dst[:, 2 + ts * TC:2 + (ts + 1) * TC],
                                    tmp[:], g[:, k:k + 1], b[:, k:k + 1],
                                    AOT.mult, AOT.add)

        stats(0)
        m0, v0 = chain_mv(pssA[0:1, 0:TC], psqA[0:1, 0:TC], TC)
        (bc0,) = chain_fin([(m0, v0, TC)])
        stats(1)
        pssh = pst_((1, 1)); psqh = pst_((1, 1))
        for k in range(NK):
            hs = src_halo(k)
            hsq = sc((P, 1), bf16)
            nc.scalar.square(hsq[:], hs)
            nc.tensor.matmul(pssh[:], ONES_KB[:], hs,
                             start=(k == 0), stop=(k == NK - 1))
            nc.tensor.matmul(psqh[:], ONES_KB[:], hsq[:],
                             start=(k == 0), stop=(k == NK - 1))
        m1, v1 = chain_mv(pssA[0:1, TC:2 * TC], psqA[0:1, TC:2 * TC], TC)
        mh, vh = chain_mv(pssh[:], psqh[:], 1)
        bc1, bch = chain_fin([(m1, v1, TC), (mh, vh, 1)])
        # ts=0 normalize (no halo dependence)
        for k in range(NK):
            norm(k, 0, *bc0)
        # halo normalize (before the ts=0 mixes, which read col 1)
        for k in range(NK):
            dst = dst_tiles[k]
            tmp = sc((P, 1), bf16)
            nc.vector.tensor_mul(tmp[:], src_halo(k), bch[0][:])
            nc.vector.tensor_add(tmp[:], tmp[:], bch[1][:])
            nc.vector.tensor_scalar(dst[:, 1:2], tmp[:], g[:, k:k + 1],
                                    b[:, k:k + 1], AOT.mult, AOT.add)
            if halo_mask:
                nc.vector.tensor_scalar(dst[:, 1:2], dst[:, 1:2],
                                        SMB[:, 0:1], None, AOT.mult)
        if post_norm is not None:
            for k in range(NK):
                post_norm(k, 0)
        for k in range(NK):
            norm(k, 1, *bc1)
            if post_norm is not None:
                post_norm(k, 1)

    def mixf(dst, h, cf, k):
        """dst[:, 0:TL] = h_shift + cf[k]*(h - h_shift), full width."""
        dh = sc((P, TL), bf16)
        nc.vector.tensor_sub(dh[:], h[:, 2:2 + TL], h[:, 1:1 + TL])
        nc.vector.scalar_tensor_tensor(dst[:, 0:TL], dh[:], cf[:, k:k + 1],
                                       h[:, 1:1 + TL], AOT.mult, AOT.add)

    # ---------- GEMM helper: out[m] = sum_k w_tiled[.,m,k] ^T @ in[k] ----------
    def gemm_std(wt_dram, in_tiles, out_tiles, n_out, act=None, accum=False,
                 n_in=NK, G=8, col_base=0, post=None, pre=None,
                 ts_major=False):
        """Host-pretiled weights: blob cols ordered (mg, k, 256). m-groups
        of 2; per group ceil(n_in/G) blob DMAs of [128, G*256]; two 2-bank
        PSUM tiles (one per mi), double-buffered across groups.
        act: None -> copy; 'sigmoid'; 'sqrelu'. accum: add into out.
        post(m, pq): custom finalize; pre(mgl): prefetch hook."""
        nh = (n_in + G - 1) // G
        for mgl in range(n_out // 2):
            if pre is not None:
                pre(mgl)
            base = col_base + mgl * n_in * 2 * P
            wtl = []
            for hf in range(nh):
                kn = min(G, n_in - hf * G)
                w = wt_tile(kn * 2 * P)
                nc.sync.dma_start(
                    w[:], wt_dram[:, base + hf * G * 2 * P:
                                  base + (hf * G + kn) * 2 * P])
                wtl.append(w)
            pq = [psa_() for _ in range(2)]
            if ts_major:
                # all ts=0 matmuls before any ts=1: lets the GEMM start as
                # soon as ts=0 inputs exist (LN still normalizing ts=1)
                for ts in range(TS):
                    for k in range(n_in):
                        w = wtl[k // G]
                        co = (k % G) * 2 * P
                        for mi in range(2):
                            nc.tensor.matmul(
                                pq[mi][:, ts * TC:(ts + 1) * TC],
                                w[:, co + mi * P:co + (mi + 1) * P],
                                in_tiles[k][:, ts * TC:(ts + 1) * TC],
                                start=(k == 0), stop=(k == n_in - 1))
            else:
                for k in range(n_in):
                    w = wtl[k // G]
                    co = (k % G) * 2 * P
                    for mi in range(2):
                        for ts in range(TS):
                            nc.tensor.matmul(
                                pq[mi][:, ts * TC:(ts + 1) * TC],
                                w[:, co + mi * P:co + (mi + 1) * P],
                                in_tiles[k][:, ts * TC:(ts + 1) * TC],
                                start=(k == 0), stop=(k == n_in - 1))
            for mi in range(2):
                m = mgl * 2 + mi
                if post is not None:
                    post(m, pq[mi])
                    continue
                dst = out_tiles[m][:, 0:TL]
                src = pq[mi][0:P, 0:TL]
                if act == "sigmoid":
                    nc.scalar.activation(dst, src, AFT.Sigmoid)
                elif act == "sqrelu":
                    rl = xsc()
                    nc.scalar.activation(rl[:, 0:TL], src, AFT.Relu)
                    nc.vector.tensor_mul(dst, rl[:, 0:TL], rl[:, 0:TL])
                elif accum:
                    nc.vector.tensor_add(dst, dst, src)
                else:
                    nc.vector.tensor_copy(dst, src)

    # ================= phase 1: LN1 =================
    # x streamed once as a host-pre-cast bf16 copy (xTb); stats + normalize
    # both read it. K-mix issues inline per (k, ts) via post_norm.
    H1 = [bigt(cols=2 + TL) for _ in range(NK)]
    MQ = [bigt(cols=TL) for _ in range(NK)]

    def post1(k, ts):
        dh = sc((P, TC), bf16)
        nc.vector.tensor_sub(dh[:], H1[k][:, 2 + ts * TC:2 + (ts + 1) * TC],
                             H1[k][:, 1 + ts * TC:1 + (ts + 1) * TC])
        nc.vector.scalar_tensor_tensor(MQ[k][:, ts * TC:(ts + 1) * TC],
                                       dh[:], MXK[:, k:k + 1],
                                       H1[k][:, 1 + ts * TC:1 + (ts + 1) * TC],
                                       AOT.mult, AOT.add)

    ln_pass(lambda k, ts: XB[k][:, 2 + ts * TC:2 + (ts + 1) * TC],
            lambda k: XB[k][:, 1:2], LN1G, LN1B, H1, halo_mask=True,
            post_norm=post1)
    XB = None

    # ================= phase 2: K / V projections =================
    KT = [bigt(cols=TL) for _ in range(NK)]
    gemm_std(params["wk_t"], MQ, KT, NK)
    for k in range(NK):
        mixf(MQ[k], H1[k], MXV, k)
    # V transposed: VT[tslab (8 x 128 tokens)] as 2 tiles of [P, 1024] each.
    # wv_t blob cols ordered (cb, k, 512); stationary = MQ slab, moving = w.
    VT = [[bigt(cols=TL) for _ in range(2)] for _ in range(2 * 4)]
    for tg in range(2):       # t-slab groups of 4
        for cb in range(4):   # c_out banks of 512
            pv = [psa_() for _ in range(2)]
            for kq in range(4):
                w = wt_tile()
                nc.sync.dma_start(
                    w[:], params["wv_t"][:, (cb * NK + kq * 4) * TC:
                                         (cb * NK + kq * 4 + 4) * TC])
                for kk in range(4):
                    k = kq * 4 + kk
                    for ti in range(4):
                        tslab = tg * 4 + ti
                        nc.tensor.matmul(
                            pv[ti // 2][:, (ti % 2) * TC:(ti % 2 + 1) * TC],
                            MQ[k][:, tslab * P:(tslab + 1) * P],
                            w[:, kk * TC:(kk + 1) * TC],
                            start=(k == 0), stop=(k == NK - 1))
            for ti in range(4):
                nc.vector.tensor_copy(
                    VT[tg * 4 + ti][cb // 2][:, (cb % 2) * TC:(cb % 2 + 1) * TC],
                    pv[ti // 2][:, (ti % 2) * TC:(ti % 2 + 1) * TC])

    def vsl(i, j, h):
        """[P,S] value slice: chunk i, 128-token slab j, head h."""
        voff = h * S
        return VT[i * 4 + j][voff // TL][:, voff % TL:voff % TL + S]

    # ============ phase 3: state contributions + AllGather ============
    # K transposed per 128-token block with ONE full 128x128 transpose
    # covering both heads of the pair.
    CONTRIB0 = [pers.tile([P, S], f32, tag=f"c0_{p}", name=f"c0_{p}")
                for p in range(NP)]
    for p in range(NP):
        wkct = []
        for hh in range(2):
            h = 2 * p + hh
            t = sc((P, 4 * S), bf16)
            nc.scalar.activation(t[:], IOTAW[:], AFT.Exp,
                                 scale=LNW[:, h:h + 1])
            wkct.append(t)
        cts = []
        for i in range(NCH):
            ptrf = pst_((P, 4 * P), bf16)
            for j in range(4):
                nc.tensor.transpose(
                    ptrf[:, j * P:(j + 1) * P],
                    KT[p][:, i * TC + j * P:i * TC + (j + 1) * P],
                    IDENT[:])
            kw = sc((P, 4 * P), bf16)
            for j in range(4):
                for hh in range(2):
                    o = j * P + hh * S
                    nc.vector.tensor_mul(kw[:, o:o + S], ptrf[:, o:o + S],
                                         wkct[hh][:, j * S:(j + 1) * S])
            pst = pst_((P, S))
            for hh in range(2):
                h = 2 * p + hh
                pr = slice(hh * S, hh * S + S)
                for j in range(4):
                    nc.tensor.matmul(pst[pr, :], kw[:, j * P + hh * S:
                                                    j * P + hh * S + S],
                                     vsl(i, j, h),
                                     start=(j == 0), stop=(j == 3))
            if i == 0:
                nc.vector.tensor_copy(CONTRIB0[p][:], pst[:])
                cts.append(CONTRIB0[p])
            else:
                c1 = ssc()
                nc.vector.tensor_copy(c1[:], pst[:])
                cts.append(c1)
        so = ssc()
        nc.vector.scalar_tensor_tensor(so[:], cts[0][:], WSPP[:, p:p + 1],
                                       cts[1][:], AOT.mult, AOT.add)
        nc.sync.dma_start(sout_d[:, p * S:(p + 1) * S], so[:])
    nc.gpsimd.collective_compute("AllGather", AOT.bypass, replica_groups=GROUPS,
                                 ins=[sout_d.opt()], outs=[sgat_d.opt()])

    # ================= phase 4: R projection (overlaps AG) =================
    for k in range(NK):
        mixf(MQ[k], H1[k], MXR, k)
    H1 = None
    RT = [bigt(cols=TL) for _ in range(NK)]
    gemm_std(params["wr_t"], MQ, RT, NK)
    MQ = None

    # incoming state = smask * (rank0 shard of gather)
    SIN = pers.tile([P, NP * S], f32, tag="sin", name="sin")
    nc.sync.dma_start(SIN[:], sgat_d[0:P, :])
    nc.vector.tensor_scalar(SIN[:], SIN[:], SMB[:, 0:1], None, AOT.mult)

    # ================= phase 5: attention =================
    # Decay mask per head = column-shifted view of M_h[p,c]=w^(c-p) (c>=p)
    # plus diagonal-block D_h[p,q]=w^(q-p-1) (q>p) + u_h*I. Mask is zero
    # for t <= jP (except diag), so pa2/pout matmuls are restricted to
    # cols >= jP. Groupnorm stats are collected per (pair, chunk) into a
    # batched [64, TC] tile; one scalar chain per chunk; normalization
    # applied with gpsimd partition-broadcasts (no tensor-engine work).
    XA = [bigt(cols=TL) for _ in range(NK)]
    PSALL = {}

    def attn_pair(i, p, masks=None):
        h0, h1 = 2 * p, 2 * p + 1
        if masks is None:
            masks = build_masks(p)
        WD_ = masks
        wb = sc(dtype=bf16)
        nc.scalar.activation(wb[:], IOTA_T[:], AFT.Exp,
                             scale=LNWPP[:, p:p + 1])
        st_mm = ssc((P, S), bf16)
        if i == 0:
            nc.vector.tensor_copy(st_mm[:], SIN[:, p * S:(p + 1) * S])
        else:
            stt = ssc()
            nc.vector.scalar_tensor_tensor(stt[:], SIN[:, p * S:(p + 1) * S],
                                           WSPP[:, p:p + 1], CONTRIB0[p][:],
                                           AOT.mult, AOT.add)
            nc.vector.tensor_copy(st_mm[:], stt[:])
        rtw = sc(dtype=bf16)
        nc.vector.tensor_mul(rtw[:], RT[p][:, i * TC:(i + 1) * TC], wb[:])
        pout = pst_((P, TC))

        def pout_mms(j, ast):
            for hh in range(2):
                h = 2 * p + hh
                pr = slice(hh * S, hh * S + S)
                nc.tensor.matmul(pout[pr, j * P:TC], vsl(i, j, h),
                                 ast[:, hh * TC + j * P:(hh + 1) * TC],
                                 start=False, stop=(j == 3))

        prev = None
        for j in range(4):
            pa2 = psa_()
            for hh in range(2):
                pr = slice(hh * S, hh * S + S)
                nc.tensor.matmul(
                    pa2[:, hh * TC + j * P:(hh + 1) * TC],
                    KT[p][pr, i * TC + j * P:i * TC + (j + 1) * P],
                    RT[p][pr, i * TC + j * P:(i + 1) * TC],
                    start=True, stop=True)
            if j == 0:
                # state term (start=True clears the pout bank); issued
                # after the first pa2 so the tensor queue isn't blocked
                # waiting on wb/rtw from the scalar/vector engines.
                for hh in range(2):
                    pr = slice(hh * S, hh * S + S)
                    nc.tensor.matmul(pout[pr, :], st_mm[pr, :], rtw[pr, :],
                                     start=True, stop=False)
            ast = sc((P, 2 * TC), bf16)
            nc.vector.tensor_mul(
                ast[:].rearrange("p (h t) -> p h t", h=2)[:, :, j * P:TC],
                pa2[:].rearrange("p (h t) -> p h t", h=2)[:, :, j * P:TC],
                WD_[:].rearrange("p (h t) -> p h t", h=2)[:, :, 0:TC - j * P])
            if prev is not None:
                pout_mms(*prev)
            prev = (j, ast)
        pout_mms(*prev)
        # raw attention out + gathered groupnorm stats (GSEL accumulate);
        # psum copy + square on the scalar engine (vector is the pacer here)
        nc.scalar.copy(XA[p][:, i * TC:(i + 1) * TC], pout[:])
        sq = sc((P, TC), bf16)
        nc.scalar.square(sq[:], XA[p][:, i * TC:(i + 1) * TC])
        if p == 0:
            PSALL[i] = psa_()
        sel = GSEL[:, 30 - 2 * p:62 - 2 * p]
        nc.tensor.matmul(PSALL[i][0:H, 0:TC], sel,
                         XA[p][:, i * TC:(i + 1) * TC],
                         start=(p == 0), stop=(p == NP - 1))
        nc.tensor.matmul(PSALL[i][0:H, TC:2 * TC], sel, sq[:],
                         start=(p == 0), stop=(p == NP - 1))

    def build_masks(p):
        # both heads' decay masks in one [P, 2TC] tile (head hh at cols
        # hh*TC) so each j-block's mask multiply is a single 3D-AP op
        wd = mt_tile()
        for hh in range(2):
            h = 2 * p + hh
            nc.scalar.activation(wd[:, hh * TC:(hh + 1) * TC], IOTA_WD[:],
                                 AFT.Exp, scale=LNW[:, h:h + 1])
            # diagonal 'u' bonus lands in the first 128-col block
            nc.vector.scalar_tensor_tensor(wd[:, hh * TC:hh * TC + P],
                                           IDENT[:], UU[:, h:h + 1],
                                           wd[:, hh * TC:hh * TC + P],
                                           AOT.mult, AOT.add)
        return wd

    def gn_chain(i):
        ps = PSALL.pop(i)
        m_ = sc((H, TC))
        nc.scalar.mul(m_[:], ps[0:H, 0:TC], 1.0 / (S * HS_DIV))
        q_ = sc((H, TC))
        nc.scalar.mul(q_[:], ps[0:H, TC:2 * TC],
                      1.0 / (S * HS_DIV * HS_DIV))
        msq = sc((H, TC)); nc.vector.tensor_mul(msq[:], m_[:], m_[:])
        var = sc((H, TC)); nc.vector.tensor_sub(var[:], q_[:], msq[:])
        lnv = sc((H, TC))
        nc.scalar.activation(lnv[:], var[:], AFT.Ln, bias=EPSB[0:H, 0:1])
        rs = sc((H, TC))
        nc.scalar.activation(rs[:], lnv[:], AFT.Exp, scale=-0.5)
        mrs = sc((H, TC))
        nc.vector.scalar_tensor_tensor(mrs[:], m_[:], -1.0, rs[:],
                                       AOT.mult, AOT.mult)
        rsh = sc((H, TC))
        nc.vector.tensor_scalar_mul(rsh[:], rs[:], 1.0 / HS_DIV)
        rsb = sc((H, TC), bf16)
        nc.vector.tensor_copy(rsb[:], rsh[:])
        mrb = sc((H, TC), bf16)
        nc.vector.tensor_copy(mrb[:], mrs[:])
        nc.sync.dma_start(rs_d[i][:], rsb[:])
        nc.sync.dma_start(mr_d[i][:], mrb[:])

    def gn_finish(i, p):
        # per-head broadcast of the groupnorm scale/shift rows via a DMA
        # bounce through DRAM (engines can't read non-32-aligned rows).
        brs = sc(dtype=bf16)
        bmrs = sc(dtype=bf16)
        for hh in range(2):
            r = 2 * p + hh
            pr = slice(hh * S, hh * S + S)
            nc.sync.dma_start(brs[pr, :],
                              rs_d[i][r:r + 1, :].partition_broadcast(S))
            nc.sync.dma_start(bmrs[pr, :],
                              mr_d[i][r:r + 1, :].partition_broadcast(S))
        xa = sc(dtype=bf16)
        nc.vector.tensor_mul(xa[:], XA[p][:, i * TC:(i + 1) * TC], brs[:])
        nc.vector.tensor_add(xa[:], xa[:], bmrs[:])
        nc.vector.tensor_scalar(XA[p][:, i * TC:(i + 1) * TC], xa[:],
                                LNXG[:, p:p + 1], LNXB[:, p:p + 1],
                                AOT.mult, AOT.add)

    for p in range(NP):
        attn_pair(0, p)
    mk0 = build_masks(0)
    mk1 = build_masks(1)
    attn_pair(1, 0, mk0)
    attn_pair(1, 1, mk1)
    gn_chain(0)
    for p in range(2, NP):
        attn_pair(1, p)
        gn_finish(0, p - 2)
    gn_finish(0, NP - 2)
    gn_finish(0, NP - 1)
    gn_chain(1)
    for p in range(NP):
        gn_finish(1, p)
    RT = KT = VT = None

    # ================= phase 6: Wo + residual, spill x' =================
    XP = [bigt(cols=TL) for _ in range(NK)]
    xr_tiles = {}

    def wo_pre(mgl):
        for m in (2 * mgl, 2 * mgl + 1):
            t = xsc()
            nc.sync.dma_start(t[:], xT[m * P:(m + 1) * P, :])
            xr_tiles[m] = t

    def wo_post(m, pq):
        xr = xr_tiles.pop(m)
        for ts in range(TS):
            c0, c1 = ts * TC, (ts + 1) * TC
            nc.vector.tensor_add(XP[m][:, c0:c1], pq[0:P, c0:c1],
                                 xr[:, 1 + c0:1 + c1])
            nc.sync.dma_start(xprime_d[m * P:(m + 1) * P, c0:c1],
                              XP[m][:, c0:c1])
        lc = sc((P, 1))
        nc.vector.tensor_copy(lc[:], XP[m][:, TL - 1:TL])
        nc.sync.dma_start(xcol_d[:, m:m + 1], lc[:])

    gemm_std(params["wo_t"], XA, XP, NK, post=wo_post, pre=wo_pre)
    XA = None
    nc.gpsimd.collective_compute("AllGather", AOT.bypass, replica_groups=GROUPS,
                                 ins=[xcol_d.opt()], outs=[xcgat_d.opt()])

    # ================= phase 7: LN2 + mixes =================
    XCH = const.tile([P, NK], f32, tag="xch")   # per-chunk halo cols
    nc.sync.dma_start(XCH[:], xcgat_d[0:P, :])
    XCHB = const.tile([P, NK], bf16, tag="xchb")
    nc.vector.tensor_copy(XCHB[:], XCH[:])

    H2 = [bigt(cols=2 + TL) for _ in range(NK)]
    MFK = [bigt(cols=TL) for _ in range(NK)]
    MFR = [bigt(cols=TL) for _ in range(NK)]

    def post2(k, ts):
        dh = sc((P, TC), bf16)
        nc.vector.tensor_sub(dh[:], H2[k][:, 2 + ts * TC:2 + (ts + 1) * TC],
                             H2[k][:, 1 + ts * TC:1 + (ts + 1) * TC])
        hsv = H2[k][:, 1 + ts * TC:1 + (ts + 1) * TC]
        nc.vector.scalar_tensor_tensor(MFK[k][:, ts * TC:(ts + 1) * TC],
                                       dh[:], FMK[:, k:k + 1], hsv,
                                       AOT.mult, AOT.add)
        nc.vector.scalar_tensor_tensor(MFR[k][:, ts * TC:(ts + 1) * TC],
                                       dh[:], FMR[:, k:k + 1], hsv,
                                       AOT.mult, AOT.add)

    ln_pass(lambda k, ts: XP[k][:, ts * TC:(ts + 1) * TC],
            lambda k: XCHB[:, k:k + 1], LN2G, LN2B, H2, halo_mask=True,
            post_norm=post2)
    XP = None
    H2 = None

    # ================= phase 8: gate = sigmoid(mfr @ wfr) =================
    GT = [bigt(cols=TL) for _ in range(NK)]
    gemm_std(params["wfr_t"], MFR, GT, NK, act="sigmoid")
    MFR = None

    # ========== phase 9: FFN quarters: kf=relu^2(mfk@wfk); kv+=wfv^T@kf ==========
    # last quarter's Wfv finalize writes y = x' + gate*kv directly.
    KV = [bigt(cols=TL) for _ in range(NK)]
    xp_tiles = {}

    def y_pre(mgl):
        for m in (2 * mgl, 2 * mgl + 1):
            t = sc((P, TL), bf16)
            nc.sync.dma_start(t[:], xprime_d[m * P:(m + 1) * P, :])
            xp_tiles[m] = t

    def y_post(m, pq):
        xp = xp_tiles.pop(m)
        for ts in range(TS):
            c0, c1 = ts * TC, (ts + 1) * TC
            kvf = sc()
            nc.vector.tensor_add(kvf[:], KV[m][:, c0:c1], pq[0:P, c0:c1])
            gk = sc()
            nc.vector.tensor_mul(gk[:], GT[m][:, c0:c1], kvf[:])
            yo = sc()
            nc.vector.tensor_add(yo[:], xp[:, c0:c1], gk[:])
            nc.sync.dma_start(yT[m * P:(m + 1) * P, c0:c1], yo[:])

    for q in range(NQ):
        KF = [bigt(cols=TL) for _ in range(JQ)]
        gemm_std(params["wfk_t"], MFK, KF, JQ, act="sqrelu",
                 col_base=q * (JQ // 2) * NK * 2 * P)
        # kv partial: contract the quarter's 14 j-chunks
        if q < NQ - 1:
            gemm_std(params["wfv_t"], KF, KV, NK, accum=(q > 0), n_in=JQ, G=7,
                     col_base=q * (NK // 2) * JQ * 2 * P)
        else:
            gemm_std(params["wfv_t"], KF, KV, NK, n_in=JQ, G=7,
                     col_base=q * (NK // 2) * JQ * 2 * P,
                     post=y_post, pre=y_pre)
        KF = None
    MFK = None

    for c in reversed(ctxs):
        c.__exit__(None, None, None)


# ----------------------------------------------------------------------
# Host-side sharding / gather
# ----------------------------------------------------------------------
import ml_dtypes

_NC_CACHE = {}


def _vec_pk(v, nk=NK):
    return np.ascontiguousarray(np.asarray(v).reshape(nk, P).T.astype(np.float32))


def _make_in_maps(inputs):
    x = np.asarray(inputs["x"], np.float32)
    bf = ml_dtypes.bfloat16
    td = np.asarray(inputs["time_decay"], np.float64)
    w = np.exp(-np.exp(td))                      # [H]
    ws = w ** TC
    wspp = np.zeros((P, NP), np.float32)
    lnwpp = np.zeros((P, NP), np.float32)
    lnw = -np.exp(td)
    for p in range(NP):
        wspp[0:S, p] = ws[2 * p]
        wspp[S:P, p] = ws[2 * p + 1]
        lnwpp[0:S, p] = lnw[2 * p]
        lnwpp[S:P, p] = lnw[2 * p + 1]
    def _tile_mk(W):
        """[n_in*128, n_mg*256] -> [128, n_mg*n_in*256], cols (mg, k, c)."""
        n_in = W.shape[0] // P
        n_mg = W.shape[1] // (2 * P)
        return np.ascontiguousarray(
            W.reshape(n_in, P, n_mg, 2 * P).transpose(1, 2, 0, 3)
             .reshape(P, -1).astype(bf))

    wcache = {}
    for nm, key in [("wr_t", "Wr"), ("wk_t", "Wk"), ("wo_t", "Wo"),
                    ("wfr_t", "Wfr"), ("wfk_t", "Wfk")]:
        wcache[nm] = _tile_mk(np.asarray(inputs[key], np.float32))
    Wfv = np.asarray(inputs["Wfv"], np.float32)
    wcache["wfv_t"] = np.ascontiguousarray(np.concatenate(
        [_tile_mk(Wfv[q * JQ * P:(q + 1) * JQ * P, :]) for q in range(NQ)],
        axis=1))
    Wv = np.asarray(inputs["Wv"], np.float32)
    wcache["wv_t"] = np.ascontiguousarray(
        Wv.reshape(NK, P, 4, TC).transpose(1, 2, 0, 3).reshape(P, -1).astype(bf))
    maps = []
    for c in range(8):
        b, half = c // 2, c % 2
        t0 = half * TL
        xh = np.zeros((C, 1 + TL), np.float32)
        xh[:, 1:] = x[b, t0:t0 + TL, :].T
        if half == 1:
            xh[:, 0] = x[b, t0 - 1, :]
        xhb = np.zeros((C, 2 + TL), np.float32)
        xhb[:, 1:] = xh
        maps.append({
            "xT": np.ascontiguousarray(xh),
            "xTb": np.ascontiguousarray(xhb.astype(bf)),
            **wcache,
            "wspp": wspp, "lnwpp": lnwpp,
            "smask": np.full((1, 1), float(half), np.float32),
            "ln1g": _vec_pk(inputs["ln1_g"]), "ln1b": _vec_pk(inputs["ln1_b"]),
            "ln2g": _vec_pk(inputs["ln2_g"]), "ln2b": _vec_pk(inputs["ln2_b"]),
            "mxk": _vec_pk(inputs["att_mix_k"]), "mxv": _vec_pk(inputs["att_mix_v"]),
            "mxr": _vec_pk(inputs["att_mix_r"]),
            "fmk": _vec_pk(inputs["ffn_mix_k"]), "fmr": _vec_pk(inputs["ffn_mix_r"]),
            "lnxg": _vec_pk(inputs["lnx_g"], NP),
            "lnxb": _vec_pk(inputs["lnx_b"], NP),
            "tdv": np.ascontiguousarray(np.asarray(inputs["time_decay"],
                                                   np.float32)[None, :]),
            "uv": np.ascontiguousarray(np.asarray(inputs["time_faaaa"],
                                                  np.float32)[None, :]),
        })
    return maps


def run_on_hw(inputs, trace=False):
    from concourse.bass_utils import run_bass_kernel_spmd
    if "nc" not in _NC_CACHE:
        _NC_CACHE["nc"] = build_nc()
    nc = _NC_CACHE["nc"]
    maps = _make_in_maps(inputs)
    res = run_bass_kernel_spmd(nc, maps, core_ids=list(range(8)), trace=trace)
    B = 4
    out = np.zeros((B, 2 * TL, C), np.float32)
    for c in range(8):
        b, half = c // 2, c % 2
        out[b, half * TL:(half + 1) * TL, :] = res.results[c]["yT"].T
    return out, res


def kernel(**inputs) -> np.ndarray:
    out, _ = run_on_hw(inputs, trace=False)
    return out
